# revision 11
# baseline (speedup 1.0000x reference)
"""Causal self-attention (B=4, T=2048, C=1024, NH=16, HS=64) on 8 trn2 cores.

Sharding: core = (batch b, head-group hg): b = core//2, hg = core%2.
Each core computes 8 heads of one batch: column-parallel W_attn (its heads'
q/k rows), row-parallel W_o (its heads' columns).  Host sums the two
head-group partials per batch and adds b_o (+ W_o @ b_v: softmax weights sum
to 1, so the v-bias contributes a constant row -> folded into the host bias).

Device algorithm (per core, all matmuls bf16 inputs / fp32 PSUM):
  q^T,k^T = W_local @ x^T          (transposed layout [j, t])
  rope via q*cosF + (P_swap @ (q*sinF_signed))   (P_swap = const permutation)
  v      = (x @ W_v^T) directly in [t, d] layout (lhsT = x^T block), with
           ones columns interleaved per head -> no PE transposes, no v bias
  S^T[k,q] = K_rot^T.T @ Q_rot^T   (scores transposed, causal blocks only,
           diagonal blocks restricted to columns >= off: no wasted PE cols)
  E = exp(S^T/8) (ScalarE, fused 1/8 scale), staircase mask on diagonal
  O^T|sums = [V|1]^T.T @ E         (fused unnormalized output + denominator,
           also column-restricted on diagonal blocks)
  O_norm = O^T * (1/sums)          (DVE recip, TensorE rank-1 broadcast)
  y = O_cat^T.T @ W_o_cols^T       (accumulate over head pairs in PSUM),
           written back as bf16 (host accumulates partials in fp32)

Perf structure (v3):
  - Two heads of a pair on partition halves 0-63/64-127: QK^T score matmuls
    run on concurrent PE row tiles; one fused [128, 2*512] exp per job.
  - Diagonal-block scores/AV matmuls only cover columns [off:]; the
    fully-masked region is never computed (saves ~37k PE cycles/core) and
    the es zero-memsets disappear.
  - V^T is computed straight from the projection (lhsT = x^T 128-block), so
    the 128 per-pair PE transposes + v bias adds of v2 are gone.
  - First weight/x DMA pieces are cb-pair sized so the first matmul starts
    ~5us in; bias rides first on the gpsimd queue; cos/sin land before the
    first rope chunk; wo on the scalar queue.
  - y stored bf16 (halves writeback bytes; host sums partials in fp32).

Head-dim channels are reordered on the host (per head: even dims then odd
dims) so RoPE pairs live in contiguous 32-partition blocks; attention scores
are invariant to this permutation since q and k use the same order, and v/W_o
stay in natural order.
"""

from contextlib import ExitStack
from itertools import chain

import numpy as np
import ml_dtypes

import concourse.bass as bass
import concourse.mybir as mybir
import concourse.tile as tile
from concourse.bass_utils import run_bass_kernel_spmd
from concourse.masks import make_identity

B, T, C = 4, 2048, 1024
NH, HS = 16, 64
P = 128
NCORES = 8
NPAIR = 4            # head pairs per core (8 local heads)
CB = C // P          # 8 contraction blocks over C
QW = 512             # q-chunk width
NTC = T // QW        # 4 q-chunks
NKB = T // P         # 16 key blocks
F32 = mybir.dt.float32
BF16 = mybir.dt.bfloat16
NPBF = ml_dtypes.bfloat16
AF = mybir.ActivationFunctionType
ALU = mybir.AluOpType

_cache = {}


def _legalize_waits(nc, max_waits=1):
    """The walrus build here allows only one sync-wait command per
    instruction; move excess Tile-generated waits onto preceding
    single-wait NoOps on the same engine (same-engine program order
    makes this equivalent)."""
    n_id = [0]
    for fn in nc.m.functions:
        for blk in fn.blocks:
            out = []
            for inst in blk.instructions:
                si = inst.sync_info
                if si is not None and si.on_wait and len(si.on_wait) > max_waits:
                    waits = list(si.on_wait)
                    excess, keep = waits[:-max_waits], waits[-max_waits:]
                    for w in excess:
                        n_id[0] += 1
                        out.append(
                            mybir.InstNoOp(
                                name=f"waitsplit-{n_id[0]}",
                                engine=inst.engine,
                                bass_nofuse=True,
                                sync_info=mybir.SyncInfo(
                                    on_wait=[w], on_update=[]
                                ),
                            )
                        )
                    inst.sync_info = mybir.SyncInfo(
                        on_wait=keep, on_update=list(si.on_update)
                    )
                out.append(inst)
            blk.instructions = out
    return nc


def _build_nc():
    nc = bass.Bass(target_bir_lowering=True)
    # all large inputs are pre-swizzled on the host so every DMA reads
    # contiguous multi-KB runs per partition (small-packet strided DMAs
    # run at a fraction of queue bandwidth)
    xT_d = nc.dram_tensor("xT", [P, NTC * CB * QW], BF16, kind="ExternalInput")
    w_d = nc.dram_tensor("wqkT", [P, NPAIR * CB * 2 * P], BF16,
                         kind="ExternalInput")
    b_d = nc.dram_tensor("bqk", [P, 8], F32, kind="ExternalInput")
    wv_d = nc.dram_tensor("wvT", [P, CB * 4 * P], BF16, kind="ExternalInput")
    wo_d = nc.dram_tensor("woT", [P, NPAIR * C], BF16, kind="ExternalInput")
    cos_d = nc.dram_tensor("cosF", [P, T], BF16, kind="ExternalInput")
    sin_d = nc.dram_tensor("sinF", [P, T], BF16, kind="ExternalInput")
    psw_d = nc.dram_tensor("psw", [P, P], BF16, kind="ExternalInput")
    band_d = nc.dram_tensor("band", [P, 2 * P], BF16, kind="ExternalInput")
    y_d = nc.dram_tensor("y", [T, C], BF16, kind="ExternalOutput")

    with tile.TileContext(nc) as tc, ExitStack() as ctx:
        const = ctx.enter_context(tc.tile_pool(name="const", bufs=1))
        wpool = ctx.enter_context(tc.tile_pool(name="wpool", bufs=2))
        qkpool = ctx.enter_context(tc.tile_pool(name="qkpool", bufs=4))
        tmppool = ctx.enter_context(tc.tile_pool(name="tmppool", bufs=2))
        rotpool = ctx.enter_context(tc.tile_pool(name="rotpool", bufs=4))
        epool = ctx.enter_context(tc.tile_pool(name="epool", bufs=8))
        extpool = ctx.enter_context(tc.tile_pool(name="extpool", bufs=3))
        rcppool = ctx.enter_context(tc.tile_pool(name="rcppool", bufs=2))
        opool = ctx.enter_context(tc.tile_pool(name="opool", bufs=4))
        ypool = ctx.enter_context(tc.tile_pool(name="ypool", bufs=2))
        # PSUM budget (8 banks): qk 2x[P,2,QW]=4, av 2x[65,QW]=2, misc 2x[P,QW]=2
        ps_qk = ctx.enter_context(tc.tile_pool(name="ps_qk", bufs=2, space="PSUM"))
        ps_av = ctx.enter_context(tc.tile_pool(name="ps_av", bufs=2, space="PSUM"))
        ps_m = ctx.enter_context(tc.tile_pool(name="ps_m", bufs=2, space="PSUM"))

        # ---- loads, ordered so pair-0 compute can start within ~5us ----
        def load_w(w_sb, p, pieces=2):
            base = p * CB * 2 * P
            npc = CB // pieces
            for i in range(pieces):
                sl = slice(base + i * npc * 2 * P, base + (i + 1) * npc * 2 * P)
                nc.sync.dma_start(
                    w_sb[:, i * npc : (i + 1) * npc, :],
                    w_d[:, sl].rearrange("p (cb j) -> p cb j", cb=npc),
                )

        w0_sb = wpool.tile([P, CB, 2 * P], BF16, tag="w", name="w_sb")
        load_w(w0_sb, 0, pieces=4)

        xT_sb = []
        for tc_i in range(NTC):
            xT_sb.append(const.tile([P, CB, QW], BF16, name=f"xT{tc_i}",
                                    tag=f"xT{tc_i}"))

        def load_x(tc_i, eng, pieces=2):
            xt = xT_sb[tc_i]
            base = tc_i * CB * QW
            npc = CB // pieces
            for i in range(pieces):
                sl = slice(base + i * npc * QW, base + (i + 1) * npc * QW)
                eng.dma_start(
                    xt[:, i * npc : (i + 1) * npc, :],
                    xT_d[:, sl].rearrange("p (cb q) -> p cb q", cb=npc),
                )

        # scalar queue: xT0 (fine pieces), xT2, wo (needed late)
        load_x(0, nc.scalar, pieces=4)
        # gpsimd queue: bias (tiny, needed by first proj output), xT1,
        # psw+cos+sin (first rope chunk ~18us), xT3, band (first diag exp)
        bias_sb = const.tile([P, 8], F32)
        nc.gpsimd.dma_start(bias_sb[:], b_d[:])
        load_x(1, nc.gpsimd, pieces=2)
        load_x(2, nc.scalar, pieces=2)
        psw_sb = const.tile([P, P], BF16)
        nc.gpsimd.dma_start(psw_sb[:], psw_d[:])
        cos_sb = const.tile([P, T], BF16)
        sin_sb = const.tile([P, T], BF16)
        nc.gpsimd.dma_start(cos_sb[:, : T // 2], cos_d[:, : T // 2])
        nc.gpsimd.dma_start(sin_sb[:, : T // 2], sin_d[:, : T // 2])
        # sync queue (after pair-0 qk weights): v weights for the direct
        # v^T projection (first vt block needs all cb of wv for tb 0)
        wv_sb = const.tile([P, CB, 4 * P], BF16)
        for i in range(2):
            sl = slice(i * 4 * 4 * P, (i + 1) * 4 * 4 * P)
            nc.sync.dma_start(
                wv_sb[:, i * 4 : (i + 1) * 4, :],
                wv_d[:, sl].rearrange("p (cb j) -> p cb j", cb=4),
            )
        load_x(3, nc.gpsimd, pieces=2)
        nc.gpsimd.dma_start(cos_sb[:, T // 2 :], cos_d[:, T // 2 :])
        nc.gpsimd.dma_start(sin_sb[:, T // 2 :], sin_d[:, T // 2 :])
        band_sb = const.tile([P, 2, P], BF16)
        nc.gpsimd.dma_start(band_sb[:],
                            band_d.rearrange("p (g q) -> p g q", g=2))
        wo_sb = const.tile([P, NPAIR, C], BF16)
        nc.scalar.dma_start(wo_sb[:], wo_d.rearrange("p (pr o) -> p pr o",
                                                     pr=NPAIR))
        ident = const.tile([P, P], BF16)
        make_identity(nc, ident[:])
        ones64 = const.tile([65, HS], BF16)
        nc.gpsimd.memset(ones64[64:65, :], 1.0)

        # v in [t, d] layout: vn[:, kb, p, h, 0:64] = v dims, [..., 64] = 1.0
        # (interleaved ones columns keep each head's [P, 65] lhsT contiguous)
        vn = const.tile([P, NKB, NPAIR, 2, HS + 1], BF16, name="vn")
        nc.gpsimd.memset(vn[:, :, :, :, HS : HS + 1], 1.0)

        ocat = [opool.tile([P, T], BF16, name=f"ocat{p}", tag="ocat")
                for p in range(NPAIR)]

        prep_out = {}

        def prep_stream(p, w_sb):
            """q/k projection + rope for pair p, chunk-major (each x chunk
            is fully consumed before the next is touched, which keeps early
            PE demand under the DMA feed rate).  Yields between PE-sized
            chunks; emission order sets scheduler priority so this work
            fills pair p-1's attention exp-wait bubbles."""
            qk = [qkpool.tile([P, T], BF16, tag="qkT", name="qkT")
                  for _ in range(2)]
            rots = [rotpool.tile([P, T], BF16, tag="rot", name="rot")
                    for _ in range(2)]
            for tc_i in range(NTC):
                sl = slice(tc_i * QW, (tc_i + 1) * QW)
                for jb in range(2):
                    dst = qk[jb]
                    bias_bc = bias_sb[
                        :, 2 * p + jb : 2 * p + jb + 1
                    ].to_broadcast((P, QW))
                    psum = ps_m.tile([P, QW], F32, tag="m", name="pj")
                    for cb in range(CB):
                        nc.tensor.matmul(
                            psum[:],
                            lhsT=w_sb[:, cb, jb * P : (jb + 1) * P],
                            rhs=xT_sb[tc_i][:, cb, :],
                            start=(cb == 0),
                            stop=(cb == CB - 1),
                        )
                    nc.vector.tensor_tensor(
                        dst[:, sl], psum[:], bias_bc, ALU.add,
                    )
                    yield
                for jb in range(2):  # rot = t*cos + P_swap @ (t*sin_signed)
                    src, rot = qk[jb], rots[jb]
                    sq = tmppool.tile([P, QW], BF16, tag="sq", name="sq")
                    nc.vector.tensor_tensor(
                        sq[:], src[:, sl], sin_sb[:, sl], ALU.mult
                    )
                    nc.vector.tensor_tensor(
                        rot[:, sl], src[:, sl], cos_sb[:, sl], ALU.mult
                    )
                    psum = ps_m.tile([P, QW], F32, tag="m", name="sw")
                    nc.tensor.matmul(
                        psum[:],
                        lhsT=psw_sb[:],
                        rhs=sq[:],
                        start=True,
                        stop=True,
                    )
                    nc.vector.tensor_tensor(
                        rot[:, sl], rot[:, sl], psum[:], ALU.add,
                    )
                    yield
            prep_out[p] = (rots[0], rots[1])

        def vt_stream(tb_lo, tb_hi):
            """v^T for 128-row blocks tb_lo..tb_hi-1, all pairs at once:
            psum[t, (p,h,d)] = sum_c x^T[c, t-block]^T wv^T[c, (p,h,d)]."""
            for tb in range(tb_lo, tb_hi):
                tci, loc = tb // 4, tb % 4
                psum = ps_m.tile([P, QW], F32, tag="m", name="vt")
                for cb in range(CB):
                    nc.tensor.matmul(
                        psum[:],
                        lhsT=xT_sb[tci][:, cb, loc * P : (loc + 1) * P],
                        rhs=wv_sb[:, cb, :],
                        start=(cb == 0),
                        stop=(cb == CB - 1),
                    )
                nc.vector.tensor_copy(
                    vn[:, tb, :, :, :HS],
                    psum[:].rearrange("p (pr h d) -> p pr h d", pr=NPAIR, h=2),
                )
                yield

        def emit_outproj_block(tb):
            """y[tb*P:(tb+1)*P, :] = sum_p ocat_p^T @ woT_p for one 128-row
            block; emitted as soon as all pairs' ocat columns are final."""
            for oc in range(2):
                psum = ps_m.tile([P, QW], F32, tag="m", name="yp")
                for p in range(NPAIR):
                    nc.tensor.matmul(
                        psum[:],
                        lhsT=ocat[p][:, tb * P : (tb + 1) * P],
                        rhs=wo_sb[:, p, oc * QW : (oc + 1) * QW],
                        start=(p == 0),
                        stop=(p == NPAIR - 1),
                    )
                yb = ypool.tile([P, QW], BF16, tag="yb")
                nc.vector.tensor_copy(yb[:], psum[:])
                eng = nc.sync if (2 * tb + oc) % 2 == 0 else nc.gpsimd
                eng.dma_start(
                    y_d[tb * P : (tb + 1) * P, oc * QW : (oc + 1) * QW],
                    yb[:],
                )

        def attn_stream(p):
            """Attention for pair p.  Jobs are packed per (qc, kb): both
            heads' QK^T matmuls are emitted back-to-back (concurrent PE row
            tiles 0-63 / 64-127), followed by one fused exp over both heads'
            PSUM banks and the two AV accumulations.  Diagonal blocks only
            compute columns [off:]."""
            rq, rk = prep_out[p]
            ps_o_cur = {}

            def normalize(qc, heads):
                # 1/sums straight off the PSUM partition-64 sums rows (DVE
                # reciprocal: one op replaces the ln/exp ScalarE chain);
                # only the O rows are staged to SBUF (DVE can read at most
                # one PSUM operand per op)
                rcpb = rcppool.tile([65, 2, QW], BF16, tag="rcpb", name="rcpb")
                ext = extpool.tile([HS, 2, QW], F32, tag="ext", name="ext")
                hs = range(2) if heads == slice(None) else [heads]
                with nc.allow_low_precision("softmax 1/sum to bf16"):
                    for h in hs:
                        nc.vector.reciprocal(
                            rcpb[64:65, h, :], ps_o_cur[h][64:65, :]
                        )
                for h in hs:
                    nc.vector.tensor_copy(ext[:, h, :], ps_o_cur[h][:HS, :])
                for h in hs:
                    rb = ps_m.tile([HS, QW], F32, tag="m", name="rb")
                    nc.tensor.matmul(
                        rb[:],
                        lhsT=ones64[64:65, :],
                        rhs=rcpb[64:65, h, :],
                        start=True,
                        stop=True,
                    )
                    nc.vector.tensor_tensor(
                        ocat[p][h * HS : (h + 1) * HS,
                                qc * QW : (qc + 1) * QW],
                        ext[:, h, :],
                        rb[:],
                        ALU.mult,
                    )

            for qc in range(NTC):
                nkb = 4 * (qc + 1)
                for h in range(2):
                    ps_o_cur[h] = ps_av.tile([HS + 1, QW], F32, tag="av",
                                             name="ps_o")
                for kb in range(nkb):
                    off = max(0, P * (kb - 4 * qc))
                    ps_s = ps_qk.tile([P, 2, QW], F32, tag="qk", name="ps_s")
                    for h in range(2):
                        nc.tensor.matmul(
                            ps_s[:, h, off:],
                            lhsT=rk[h * HS : (h + 1) * HS,
                                    kb * P : (kb + 1) * P],
                            rhs=rq[h * HS : (h + 1) * HS,
                                   qc * QW + off : (qc + 1) * QW],
                            start=True,
                            stop=True,
                        )
                    es = epool.tile([P, 2, QW], BF16, tag="es", name="es")
                    if off == 0:
                        nc.scalar.activation(
                            es[:].rearrange("p g q -> p (g q)"),
                            ps_s[:].rearrange("p g q -> p (g q)"),
                            AF.Exp,
                            scale=0.125,
                        )
                    else:  # diagonal block: partial exp + staircase mask
                        nc.scalar.activation(
                            es[:, :, off:], ps_s[:, :, off:],
                            AF.Exp, scale=0.125,
                        )
                    if kb - 4 * qc >= 0:
                        nc.gpsimd.tensor_tensor(
                            es[:, :, off : off + P],
                            es[:, :, off : off + P],
                            band_sb[:],
                            ALU.mult,
                        )
                    # Late priority: when several PE instructions are ready
                    # the scheduler prefers QKT/prep work, so AVs trail exp
                    # by as much as the es pool depth allows (robust to
                    # cost-model vs hardware timing skew).
                    with tc.high_priority(offset=-1_000_000):
                        for h in range(2):
                            nc.tensor.matmul(
                                ps_o_cur[h][:, off:],
                                lhsT=vn[:, kb, p, h, :],
                                rhs=es[:, h, off:],
                                start=(kb == 0),
                                stop=(kb == nkb - 1),
                            )
                    yield
                # pair 3's normalize feeds the out-projection: keep it eager
                # and per-head there (shortest latency to the first rank-1);
                # defer it elsewhere (nothing reads ocat until pair 3)
                if p < NPAIR - 1:
                    with tc.high_priority(offset=-1_000_000):
                        normalize(qc, slice(None))
                else:
                    for h in range(2):
                        normalize(qc, h)
                yield
                if p == NPAIR - 1:
                    with tc.high_priority(offset=-1_000_000):
                        for tb in range(4 * qc, 4 * qc + 4):
                            emit_outproj_block(tb)
                    yield

        def drive(a_gen, b_gen, ratio=2):
            done_a = a_gen is None
            done_b = b_gen is None
            while not (done_a and done_b):
                if not done_a:
                    for _ in range(ratio):
                        try:
                            next(a_gen)
                        except StopIteration:
                            done_a = True
                            break
                if not done_b:
                    try:
                        next(b_gen)
                    except StopIteration:
                        done_b = True

        w_tiles = {0: w0_sb}
        w_tiles[1] = wpool.tile([P, CB, 2 * P], BF16, tag="w", name="w_sb")
        load_w(w_tiles[1], 1)
        for _ in prep_stream(0, w0_sb):
            pass
        for _ in vt_stream(0, 4):
            pass
        for p in range(NPAIR):
            if p + 1 < NPAIR:
                if p + 2 < NPAIR:
                    w_tiles[p + 2] = wpool.tile([P, CB, 2 * P], BF16,
                                                tag="w", name="w_sb")
                    load_w(w_tiles[p + 2], p + 2)
                companion = prep_stream(p + 1, w_tiles[p + 1])
                if p == 0:
                    companion = chain(vt_stream(4, NKB), companion)
            else:
                companion = None
            drive(attn_stream(p), companion, ratio=2)
    return _legalize_waits(nc)


def _rope_tables():
    inv = 1.0 / (1000.0 ** (np.arange(0, HS, 2, dtype=np.float64) / HS))
    t = np.arange(T, dtype=np.float64)[:, None] * inv[None, :]
    sinT = np.sin(t).astype(np.float32).T  # [32, T]
    cosT = np.cos(t).astype(np.float32).T
    cosF = np.concatenate([cosT] * 4, 0)  # [128, T]
    # sign layout for multiply-BEFORE-swap: sq = q*sinF, swapped(sq) lands as
    # [-v*sin; +u*sin] in the [u; v] destination slots.
    sinF = np.concatenate([sinT, -sinT, sinT, -sinT], 0)
    return cosF, sinF


def _host_prep():
    cosF, sinF = _rope_tables()
    psw = np.zeros((P, P), np.float32)
    for hh in range(2):
        o = hh * HS
        psw[o : o + 32, o + 32 : o + 64] = np.eye(32)
        psw[o + 32 : o + 64, o : o + 32] = np.eye(32)
    # band[p, j] = 1 iff j >= p: causal triangle for the 128-wide diagonal
    # band, replicated for the two packed heads
    tri = np.tril(np.ones((P, P), np.float32)).T
    band = np.concatenate([tri, tri], axis=1)  # [P, 2P]
    return cosF, sinF, psw, band


def kernel(x, W_attn, b_attn, W_o, b_o, _trace=False, _tmpdir=None):
    x = np.asarray(x, np.float32)
    W_attn = np.asarray(W_attn, np.float32)
    b_attn = np.asarray(b_attn, np.float32)
    W_o = np.asarray(W_o, np.float32)
    b_o = np.asarray(b_o, np.float32)

    if "nc" not in _cache:
        _cache["nc"] = _build_nc()
    nc = _cache["nc"]

    cosF, sinF, psw, band = _host_prep()
    cosF_b, sinF_b = cosF.astype(NPBF), sinF.astype(NPBF)
    psw_b, band_b = psw.astype(NPBF), band.astype(NPBF)

    def head_rows(h):  # q-rows of head h, evens then odds
        base = h * HS
        return np.concatenate(
            [np.arange(base, base + HS, 2), np.arange(base + 1, base + HS, 2)]
        )

    in_maps = []
    for core in range(NCORES):
        b, hg = core // 2, core % 2
        heads = [hg * 8 + i for i in range(8)]
        qk_rows = []
        v_rows = []
        for p in range(NPAIR):
            h0, h1 = heads[2 * p], heads[2 * p + 1]
            qrows = np.concatenate([head_rows(h0), head_rows(h1)])
            qk_rows += [qrows, C + qrows]
            v_rows += [2 * C + np.concatenate(
                [np.arange(h0 * HS, (h0 + 1) * HS),
                 np.arange(h1 * HS, (h1 + 1) * HS)])]
        qk_rows = np.concatenate(qk_rows)  # [1024] pair-major (q,k) order
        v_rows = np.concatenate(v_rows)    # [512]  pair-major natural order
        # device-friendly swizzles: partition-major with contiguous per-
        # partition runs ([P, ...]) so DMAs move multi-KB packets
        wqkT = (
            W_attn[qk_rows].T.reshape(CB, P, NPAIR, 2 * P)
            .transpose(1, 2, 0, 3)
            .reshape(P, NPAIR * CB * 2 * P)
        )
        wqkT = np.ascontiguousarray(wqkT).astype(NPBF)
        bqk = np.ascontiguousarray(b_attn[qk_rows].reshape(8, P).T)  # [P, 8]
        wvT = (
            W_attn[v_rows].T.reshape(CB, P, 4 * P)
            .transpose(1, 0, 2)
            .reshape(P, CB * 4 * P)
        )
        wvT = np.ascontiguousarray(wvT).astype(NPBF)
        woT = (
            W_o[:, hg * 512 : (hg + 1) * 512].T.reshape(NPAIR, P, C)
            .transpose(1, 0, 2)
            .reshape(P, NPAIR * C)
        )
        woT = np.ascontiguousarray(woT).astype(NPBF)
        xT = (
            x[b].T.reshape(CB, P, NTC, QW)
            .transpose(1, 2, 0, 3)
            .reshape(P, NTC * CB * QW)
        )
        xT = np.ascontiguousarray(xT).astype(NPBF)
        in_maps.append(
            dict(xT=xT, wqkT=wqkT, bqk=bqk, wvT=wvT, woT=woT, cosF=cosF_b,
                 sinF=sinF_b, psw=psw_b, band=band_b)
        )

    res = run_bass_kernel_spmd(nc, in_maps, core_ids=list(range(NCORES)),
                               trace=_trace, tmpdir=_tmpdir)
    y = np.zeros((B, T, C), np.float32)
    for core in range(NCORES):
        y[core // 2] += res.results[core]["y"].astype(np.float32)
    # v-bias contribution: sum_k softmax_k (v_k + b_v) = (sum) + b_v, so
    # y gains the constant row b_v @ W_o^T; fold it in with b_o here.
    b_v = b_attn[2 * C :]
    y += (W_o @ b_v + b_o)[None, None, :]
    if _trace:
        _cache["last_result"] = res
    return y


# revision 15
# speedup vs baseline: 1.2182x; 1.2182x over previous
"""Causal self-attention (B=4, T=2048, C=1024, NH=16, HS=64) on 8 trn2 cores.

Sharding: core = (batch b, head-group hg): b = core//2, hg = core%2.
Each core computes 8 heads of one batch: column-parallel W_attn (its heads'
q/k rows), row-parallel W_o (its heads' columns).  Host sums the two
head-group partials per batch and adds b_o (+ W_o @ b_v: softmax weights sum
to 1, so the v-bias contributes a constant row -> folded into the host bias).

Device algorithm (per core, all matmuls bf16 inputs / fp32 PSUM):
  q^T,k^T = W_local @ x^T          (transposed layout [j, t])
  rope via q*cosF + (P_swap @ (q*sinF_signed))   (P_swap = const permutation)
  v      = (x @ W_v^T) directly in [t, d] layout (lhsT = x^T block), with
           ones columns interleaved per head -> no PE transposes, no v bias
  S^T[k,q] = K_rot^T.T @ Q_rot^T   (scores transposed, causal blocks only,
           diagonal blocks restricted to columns >= off: no wasted PE cols)
  E = exp(S^T/8) (ScalarE, fused 1/8 scale), staircase mask on diagonal
  O^T|sums = [V|1]^T.T @ E         (fused unnormalized output + denominator,
           also column-restricted on diagonal blocks)
  O_norm = O^T * (1/sums)          (DVE recip, TensorE rank-1 broadcast)
  y = O_cat^T.T @ W_o_cols^T       (accumulate over head pairs in PSUM),
           written back as bf16 (host accumulates partials in fp32)

Perf structure (v3):
  - Two heads of a pair on partition halves 0-63/64-127: QK^T score matmuls
    run on concurrent PE row tiles; one fused [128, 2*512] exp per job.
  - Diagonal-block scores/AV matmuls only cover columns [off:]; the
    fully-masked region is never computed (saves ~37k PE cycles/core) and
    the es zero-memsets disappear.
  - V^T is computed straight from the projection (lhsT = x^T 128-block), so
    the 128 per-pair PE transposes + v bias adds of v2 are gone.
  - First weight/x DMA pieces are cb-pair sized so the first matmul starts
    ~5us in; bias rides first on the gpsimd queue; cos/sin land before the
    first rope chunk; wo on the scalar queue.
  - y stored bf16 (halves writeback bytes; host sums partials in fp32).

Head-dim channels are reordered on the host (per head: even dims then odd
dims) so RoPE pairs live in contiguous 32-partition blocks; attention scores
are invariant to this permutation since q and k use the same order, and v/W_o
stay in natural order.
"""

from contextlib import ExitStack
from itertools import chain

import numpy as np
import ml_dtypes

import concourse.bass as bass
import concourse.mybir as mybir
import concourse.tile as tile
from concourse.bass_utils import run_bass_kernel_spmd
from concourse.masks import make_identity

B, T, C = 4, 2048, 1024
NH, HS = 16, 64
P = 128
NCORES = 8
NPAIR = 4            # head pairs per core (8 local heads)
CB = C // P          # 8 contraction blocks over C
QW = 512             # q-chunk width
NTC = T // QW        # 4 q-chunks
NKB = T // P         # 16 key blocks
F32 = mybir.dt.float32
BF16 = mybir.dt.bfloat16
NPBF = ml_dtypes.bfloat16
AF = mybir.ActivationFunctionType
ALU = mybir.AluOpType

_cache = {}


def _legalize_waits(nc, max_waits=1):
    """The walrus build here allows only one sync-wait command per
    instruction; move excess Tile-generated waits onto preceding
    single-wait NoOps on the same engine (same-engine program order
    makes this equivalent)."""
    n_id = [0]
    for fn in nc.m.functions:
        for blk in fn.blocks:
            out = []
            for inst in blk.instructions:
                si = inst.sync_info
                if si is not None and si.on_wait and len(si.on_wait) > max_waits:
                    waits = list(si.on_wait)
                    excess, keep = waits[:-max_waits], waits[-max_waits:]
                    for w in excess:
                        n_id[0] += 1
                        out.append(
                            mybir.InstNoOp(
                                name=f"waitsplit-{n_id[0]}",
                                engine=inst.engine,
                                bass_nofuse=True,
                                sync_info=mybir.SyncInfo(
                                    on_wait=[w], on_update=[]
                                ),
                            )
                        )
                    inst.sync_info = mybir.SyncInfo(
                        on_wait=keep, on_update=list(si.on_update)
                    )
                out.append(inst)
            blk.instructions = out
    return nc


def _build_nc():
    nc = bass.Bass(target_bir_lowering=True)
    # all large inputs are pre-swizzled on the host so every DMA reads
    # contiguous multi-KB runs per partition (small-packet strided DMAs
    # run at a fraction of queue bandwidth)
    xT_d = nc.dram_tensor("xT", [P, NTC * CB * QW], BF16, kind="ExternalInput")
    w_d = nc.dram_tensor("wqkT", [P, NPAIR * CB * 2 * P], BF16,
                         kind="ExternalInput")
    b_d = nc.dram_tensor("bqk", [P, 8], F32, kind="ExternalInput")
    wv_d = nc.dram_tensor("wvT", [P, CB * 4 * P], BF16, kind="ExternalInput")
    wo_d = nc.dram_tensor("woT", [P, NPAIR * C], BF16, kind="ExternalInput")
    cos_d = nc.dram_tensor("cosF", [P, T], BF16, kind="ExternalInput")
    sin_d = nc.dram_tensor("sinF", [P, T], BF16, kind="ExternalInput")
    psw_d = nc.dram_tensor("psw", [P, P], BF16, kind="ExternalInput")
    band_d = nc.dram_tensor("band", [P, 2 * P], BF16, kind="ExternalInput")
    y_d = nc.dram_tensor("y", [T, C], BF16, kind="ExternalOutput")

    with tile.TileContext(nc) as tc, ExitStack() as ctx:
        const = ctx.enter_context(tc.tile_pool(name="const", bufs=1))
        wpool = ctx.enter_context(tc.tile_pool(name="wpool", bufs=2))
        qkpool = ctx.enter_context(tc.tile_pool(name="qkpool", bufs=4))
        tmppool = ctx.enter_context(tc.tile_pool(name="tmppool", bufs=2))
        rotpool = ctx.enter_context(tc.tile_pool(name="rotpool", bufs=4))
        epool = ctx.enter_context(tc.tile_pool(name="epool", bufs=8))
        extpool = ctx.enter_context(tc.tile_pool(name="extpool", bufs=3))
        rcppool = ctx.enter_context(tc.tile_pool(name="rcppool", bufs=2))
        opool = ctx.enter_context(tc.tile_pool(name="opool", bufs=4))
        ypool = ctx.enter_context(tc.tile_pool(name="ypool", bufs=2))
        # PSUM budget (8 banks): qk 2x[P,2,QW]=4, av 2x[65,QW]=2, misc 2x[P,QW]=2
        ps_qk = ctx.enter_context(tc.tile_pool(name="ps_qk", bufs=2, space="PSUM"))
        ps_av = ctx.enter_context(tc.tile_pool(name="ps_av", bufs=2, space="PSUM"))
        ps_m = ctx.enter_context(tc.tile_pool(name="ps_m", bufs=2, space="PSUM"))

        # ---- loads, ordered so pair-0 compute can start within ~5us ----
        def load_w(w_sb, p, pieces=2):
            base = p * CB * 2 * P
            npc = CB // pieces
            for i in range(pieces):
                sl = slice(base + i * npc * 2 * P, base + (i + 1) * npc * 2 * P)
                nc.sync.dma_start(
                    w_sb[:, i * npc : (i + 1) * npc, :],
                    w_d[:, sl].rearrange("p (cb j) -> p cb j", cb=npc),
                )

        w0_sb = wpool.tile([P, CB, 2 * P], BF16, tag="w", name="w_sb")
        load_w(w0_sb, 0, pieces=4)

        xT_sb = []
        for tc_i in range(NTC):
            xT_sb.append(const.tile([P, CB, QW], BF16, name=f"xT{tc_i}",
                                    tag=f"xT{tc_i}"))

        def load_x(tc_i, eng, pieces=2):
            xt = xT_sb[tc_i]
            base = tc_i * CB * QW
            npc = CB // pieces
            for i in range(pieces):
                sl = slice(base + i * npc * QW, base + (i + 1) * npc * QW)
                eng.dma_start(
                    xt[:, i * npc : (i + 1) * npc, :],
                    xT_d[:, sl].rearrange("p (cb q) -> p cb q", cb=npc),
                )

        # scalar queue: xT0 (fine pieces), xT2, wo (needed late)
        load_x(0, nc.scalar, pieces=4)
        # gpsimd queue: bias (tiny, needed by first proj output), xT1,
        # psw+cos+sin (first rope chunk ~18us), xT3, band (first diag exp)
        bias_sb = const.tile([P, 8], F32)
        nc.gpsimd.dma_start(bias_sb[:], b_d[:])
        load_x(1, nc.gpsimd, pieces=2)
        load_x(2, nc.scalar, pieces=2)
        psw_sb = const.tile([P, P], BF16)
        nc.gpsimd.dma_start(psw_sb[:], psw_d[:])
        cos_sb = const.tile([P, T], BF16)
        sin_sb = const.tile([P, T], BF16)
        nc.gpsimd.dma_start(cos_sb[:, : T // 2], cos_d[:, : T // 2])
        nc.gpsimd.dma_start(sin_sb[:, : T // 2], sin_d[:, : T // 2])
        # sync queue (after pair-0 qk weights): v weights for the direct
        # v^T projection (first vt block needs all cb of wv for tb 0)
        wv_sb = const.tile([P, CB, 4 * P], BF16)
        for i in range(2):
            sl = slice(i * 4 * 4 * P, (i + 1) * 4 * 4 * P)
            nc.sync.dma_start(
                wv_sb[:, i * 4 : (i + 1) * 4, :],
                wv_d[:, sl].rearrange("p (cb j) -> p cb j", cb=4),
            )
        load_x(3, nc.gpsimd, pieces=2)
        nc.gpsimd.dma_start(cos_sb[:, T // 2 :], cos_d[:, T // 2 :])
        nc.gpsimd.dma_start(sin_sb[:, T // 2 :], sin_d[:, T // 2 :])
        band_sb = const.tile([P, 2, P], BF16)
        nc.gpsimd.dma_start(band_sb[:],
                            band_d.rearrange("p (g q) -> p g q", g=2))
        wo_sb = const.tile([P, NPAIR, C], BF16)
        nc.scalar.dma_start(wo_sb[:], wo_d.rearrange("p (pr o) -> p pr o",
                                                     pr=NPAIR))
        ident = const.tile([P, P], BF16)
        make_identity(nc, ident[:])
        ones64 = const.tile([65, HS], BF16)
        nc.gpsimd.memset(ones64[64:65, :], 1.0)

        # v in [t, d] layout: vn[:, kb, p, h, 0:64] = v dims, [..., 64] = 1.0
        # (interleaved ones columns keep each head's [P, 65] lhsT contiguous)
        vn = const.tile([P, NKB, NPAIR, 2, HS + 1], BF16, name="vn")
        nc.gpsimd.memset(vn[:, :, :, :, HS : HS + 1], 1.0)

        ocat = [opool.tile([P, T], BF16, name=f"ocat{p}", tag="ocat")
                for p in range(NPAIR)]

        prep_out = {}

        def prep_stream(p, w_sb):
            """q/k projection + rope for pair p, chunk-major (each x chunk
            is fully consumed before the next is touched, which keeps early
            PE demand under the DMA feed rate).  Yields between PE-sized
            chunks; emission order sets scheduler priority so this work
            fills pair p-1's attention exp-wait bubbles."""
            qk = [qkpool.tile([P, T], BF16, tag="qkT", name="qkT")
                  for _ in range(2)]
            rots = [rotpool.tile([P, T], BF16, tag="rot", name="rot")
                    for _ in range(2)]
            for tc_i in range(NTC):
                sl = slice(tc_i * QW, (tc_i + 1) * QW)
                for jb in range(2):
                    dst = qk[jb]
                    bias_bc = bias_sb[
                        :, 2 * p + jb : 2 * p + jb + 1
                    ].to_broadcast((P, QW))
                    psum = ps_m.tile([P, QW], F32, tag="m", name="pj")
                    for cb in range(CB):
                        nc.tensor.matmul(
                            psum[:],
                            lhsT=w_sb[:, cb, jb * P : (jb + 1) * P],
                            rhs=xT_sb[tc_i][:, cb, :],
                            start=(cb == 0),
                            stop=(cb == CB - 1),
                        )
                    nc.vector.tensor_tensor(
                        dst[:, sl], psum[:], bias_bc, ALU.add,
                    )
                    yield
                for jb in range(2):  # rot = t*cos + P_swap @ (t*sin_signed)
                    src, rot = qk[jb], rots[jb]
                    sq = tmppool.tile([P, QW], BF16, tag="sq", name="sq")
                    nc.vector.tensor_tensor(
                        sq[:], src[:, sl], sin_sb[:, sl], ALU.mult
                    )
                    nc.vector.tensor_tensor(
                        rot[:, sl], src[:, sl], cos_sb[:, sl], ALU.mult
                    )
                    psum = ps_m.tile([P, QW], F32, tag="m", name="sw")
                    nc.tensor.matmul(
                        psum[:],
                        lhsT=psw_sb[:],
                        rhs=sq[:],
                        start=True,
                        stop=True,
                    )
                    nc.vector.tensor_tensor(
                        rot[:, sl], rot[:, sl], psum[:], ALU.add,
                    )
                    yield
            prep_out[p] = (rots[0], rots[1])

        def vt_stream(tb_lo, tb_hi):
            """v^T for 128-row blocks tb_lo..tb_hi-1, all pairs at once:
            psum[t, (p,h,d)] = sum_c x^T[c, t-block]^T wv^T[c, (p,h,d)]."""
            for tb in range(tb_lo, tb_hi):
                tci, loc = tb // 4, tb % 4
                psum = ps_m.tile([P, QW], F32, tag="m", name="vt")
                for cb in range(CB):
                    nc.tensor.matmul(
                        psum[:],
                        lhsT=xT_sb[tci][:, cb, loc * P : (loc + 1) * P],
                        rhs=wv_sb[:, cb, :],
                        start=(cb == 0),
                        stop=(cb == CB - 1),
                    )
                nc.vector.tensor_copy(
                    vn[:, tb, :, :, :HS],
                    psum[:].rearrange("p (pr h d) -> p pr h d", pr=NPAIR, h=2),
                )
                yield

        def emit_outproj_block(tb):
            """y[tb*P:(tb+1)*P, :] = sum_p ocat_p^T @ woT_p for one 128-row
            block; emitted as soon as all pairs' ocat columns are final."""
            for oc in range(2):
                psum = ps_m.tile([P, QW], F32, tag="m", name="yp")
                for p in range(NPAIR):
                    nc.tensor.matmul(
                        psum[:],
                        lhsT=ocat[p][:, tb * P : (tb + 1) * P],
                        rhs=wo_sb[:, p, oc * QW : (oc + 1) * QW],
                        start=(p == 0),
                        stop=(p == NPAIR - 1),
                    )
                yb = ypool.tile([P, QW], BF16, tag="yb")
                nc.vector.tensor_copy(yb[:], psum[:])
                eng = nc.sync if (2 * tb + oc) % 2 == 0 else nc.gpsimd
                eng.dma_start(
                    y_d[tb * P : (tb + 1) * P, oc * QW : (oc + 1) * QW],
                    yb[:],
                )

        def attn_stream(p):
            """Attention for pair p.  Jobs are packed per (qc, kb): both
            heads' QK^T matmuls are emitted back-to-back (concurrent PE row
            tiles 0-63 / 64-127), followed by one fused exp over both heads'
            PSUM banks and the two AV accumulations.  Diagonal blocks only
            compute columns [off:]."""
            rq, rk = prep_out[p]
            ps_o_cur = {}

            def normalize(ext2, qc, heads):
                # 1/sums: ln+exp on the partition-64 sums row (same ACT
                # table as the scores exp -> no table reload)
                rcpf = rcppool.tile([65, 2, QW], F32, tag="rcpf", name="rcpf")
                nc.scalar.activation(
                    rcpf[64:65, heads, :], ext2[64:65, heads, :], AF.Ln
                )
                rcpb = rcppool.tile([65, 2, QW], BF16, tag="rcpb", name="rcpb")
                nc.scalar.activation(
                    rcpb[64:65, heads, :], rcpf[64:65, heads, :],
                    AF.Exp, scale=-1.0,
                )
                hs = range(2) if heads == slice(None) else [heads]
                for h in hs:
                    rb = ps_m.tile([HS, QW], F32, tag="m", name="rb")
                    nc.tensor.matmul(
                        rb[:],
                        lhsT=ones64[64:65, :],
                        rhs=rcpb[64:65, h, :],
                        start=True,
                        stop=True,
                    )
                    nc.vector.tensor_tensor(
                        ocat[p][h * HS : (h + 1) * HS,
                                qc * QW : (qc + 1) * QW],
                        ext2[:HS, h, :],
                        rb[:],
                        ALU.mult,
                    )

            for qc in range(NTC):
                nkb = 4 * (qc + 1)
                for h in range(2):
                    ps_o_cur[h] = ps_av.tile([HS + 1, QW], F32, tag="av",
                                             name="ps_o")
                for kb in range(nkb):
                    off = max(0, P * (kb - 4 * qc))
                    ps_s = ps_qk.tile([P, 2, QW], F32, tag="qk", name="ps_s")
                    for h in range(2):
                        nc.tensor.matmul(
                            ps_s[:, h, off:],
                            lhsT=rk[h * HS : (h + 1) * HS,
                                    kb * P : (kb + 1) * P],
                            rhs=rq[h * HS : (h + 1) * HS,
                                   qc * QW + off : (qc + 1) * QW],
                            start=True,
                            stop=True,
                        )
                    es = epool.tile([P, 2, QW], BF16, tag="es", name="es")
                    if off == 0:
                        nc.scalar.activation(
                            es[:].rearrange("p g q -> p (g q)"),
                            ps_s[:].rearrange("p g q -> p (g q)"),
                            AF.Exp,
                            scale=0.125,
                        )
                    else:  # diagonal block: partial exp + staircase mask
                        nc.scalar.activation(
                            es[:, :, off:], ps_s[:, :, off:],
                            AF.Exp, scale=0.125,
                        )
                    if kb - 4 * qc >= 0:
                        nc.gpsimd.tensor_tensor(
                            es[:, :, off : off + P],
                            es[:, :, off : off + P],
                            band_sb[:],
                            ALU.mult,
                        )
                    # Late priority: when several PE instructions are ready
                    # the scheduler prefers QKT/prep work, so AVs trail exp
                    # by as much as the es pool depth allows (robust to
                    # cost-model vs hardware timing skew).
                    with tc.high_priority(offset=-1_000_000):
                        for h in range(2):
                            nc.tensor.matmul(
                                ps_o_cur[h][:, off:],
                                lhsT=vn[:, kb, p, h, :],
                                rhs=es[:, h, off:],
                                start=(kb == 0),
                                stop=(kb == nkb - 1),
                            )
                    yield
                # pair 3's normalize feeds the out-projection: keep it eager
                # and per-head there (shortest latency to the first rank-1);
                # defer it elsewhere (nothing reads ocat until pair 3)
                if p < NPAIR - 1:
                    with tc.high_priority(offset=-1_000_000):
                        ext2 = extpool.tile([HS + 1, 2, QW], F32, tag="ext",
                                            name="ext2")
                        for h in range(2):
                            nc.vector.tensor_copy(ext2[:, h, :],
                                                  ps_o_cur[h][:])
                        normalize(ext2, qc, slice(None))
                else:
                    ext2 = extpool.tile([HS + 1, 2, QW], F32, tag="ext",
                                        name="ext2")
                    for h in range(2):
                        nc.vector.tensor_copy(ext2[:, h, :], ps_o_cur[h][:])
                        normalize(ext2, qc, h)
                yield
                if p == NPAIR - 1:
                    with tc.high_priority(offset=-1_000_000):
                        for tb in range(4 * qc, 4 * qc + 4):
                            emit_outproj_block(tb)
                    yield

        def drive(a_gen, b_gen, ratio=2):
            done_a = a_gen is None
            done_b = b_gen is None
            while not (done_a and done_b):
                if not done_a:
                    for _ in range(ratio):
                        try:
                            next(a_gen)
                        except StopIteration:
                            done_a = True
                            break
                if not done_b:
                    try:
                        next(b_gen)
                    except StopIteration:
                        done_b = True

        w_tiles = {0: w0_sb}
        w_tiles[1] = wpool.tile([P, CB, 2 * P], BF16, tag="w", name="w_sb")
        load_w(w_tiles[1], 1)
        for _ in prep_stream(0, w0_sb):
            pass
        for _ in vt_stream(0, 4):
            pass
        for p in range(NPAIR):
            if p + 1 < NPAIR:
                if p + 2 < NPAIR:
                    w_tiles[p + 2] = wpool.tile([P, CB, 2 * P], BF16,
                                                tag="w", name="w_sb")
                    load_w(w_tiles[p + 2], p + 2)
                companion = prep_stream(p + 1, w_tiles[p + 1])
                if p == 0:
                    companion = chain(vt_stream(4, NKB), companion)
            else:
                companion = None
            drive(attn_stream(p), companion, ratio=2)
    return _legalize_waits(nc)


def _rope_tables():
    inv = 1.0 / (1000.0 ** (np.arange(0, HS, 2, dtype=np.float64) / HS))
    t = np.arange(T, dtype=np.float64)[:, None] * inv[None, :]
    sinT = np.sin(t).astype(np.float32).T  # [32, T]
    cosT = np.cos(t).astype(np.float32).T
    cosF = np.concatenate([cosT] * 4, 0)  # [128, T]
    # sign layout for multiply-BEFORE-swap: sq = q*sinF, swapped(sq) lands as
    # [-v*sin; +u*sin] in the [u; v] destination slots.
    sinF = np.concatenate([sinT, -sinT, sinT, -sinT], 0)
    return cosF, sinF


def _host_prep():
    cosF, sinF = _rope_tables()
    psw = np.zeros((P, P), np.float32)
    for hh in range(2):
        o = hh * HS
        psw[o : o + 32, o + 32 : o + 64] = np.eye(32)
        psw[o + 32 : o + 64, o : o + 32] = np.eye(32)
    # band[p, j] = 1 iff j >= p: causal triangle for the 128-wide diagonal
    # band, replicated for the two packed heads
    tri = np.tril(np.ones((P, P), np.float32)).T
    band = np.concatenate([tri, tri], axis=1)  # [P, 2P]
    return cosF, sinF, psw, band


def kernel(x, W_attn, b_attn, W_o, b_o, _trace=False, _tmpdir=None):
    x = np.asarray(x, np.float32)
    W_attn = np.asarray(W_attn, np.float32)
    b_attn = np.asarray(b_attn, np.float32)
    W_o = np.asarray(W_o, np.float32)
    b_o = np.asarray(b_o, np.float32)

    if "nc" not in _cache:
        _cache["nc"] = _build_nc()
    nc = _cache["nc"]

    cosF, sinF, psw, band = _host_prep()
    cosF_b, sinF_b = cosF.astype(NPBF), sinF.astype(NPBF)
    psw_b, band_b = psw.astype(NPBF), band.astype(NPBF)

    def head_rows(h):  # q-rows of head h, evens then odds
        base = h * HS
        return np.concatenate(
            [np.arange(base, base + HS, 2), np.arange(base + 1, base + HS, 2)]
        )

    in_maps = []
    for core in range(NCORES):
        b, hg = core // 2, core % 2
        heads = [hg * 8 + i for i in range(8)]
        qk_rows = []
        v_rows = []
        for p in range(NPAIR):
            h0, h1 = heads[2 * p], heads[2 * p + 1]
            qrows = np.concatenate([head_rows(h0), head_rows(h1)])
            qk_rows += [qrows, C + qrows]
            v_rows += [2 * C + np.concatenate(
                [np.arange(h0 * HS, (h0 + 1) * HS),
                 np.arange(h1 * HS, (h1 + 1) * HS)])]
        qk_rows = np.concatenate(qk_rows)  # [1024] pair-major (q,k) order
        v_rows = np.concatenate(v_rows)    # [512]  pair-major natural order
        # device-friendly swizzles: partition-major with contiguous per-
        # partition runs ([P, ...]) so DMAs move multi-KB packets
        wqkT = (
            W_attn[qk_rows].T.reshape(CB, P, NPAIR, 2 * P)
            .transpose(1, 2, 0, 3)
            .reshape(P, NPAIR * CB * 2 * P)
        )
        wqkT = np.ascontiguousarray(wqkT).astype(NPBF)
        bqk = np.ascontiguousarray(b_attn[qk_rows].reshape(8, P).T)  # [P, 8]
        wvT = (
            W_attn[v_rows].T.reshape(CB, P, 4 * P)
            .transpose(1, 0, 2)
            .reshape(P, CB * 4 * P)
        )
        wvT = np.ascontiguousarray(wvT).astype(NPBF)
        woT = (
            W_o[:, hg * 512 : (hg + 1) * 512].T.reshape(NPAIR, P, C)
            .transpose(1, 0, 2)
            .reshape(P, NPAIR * C)
        )
        woT = np.ascontiguousarray(woT).astype(NPBF)
        xT = (
            x[b].T.reshape(CB, P, NTC, QW)
            .transpose(1, 2, 0, 3)
            .reshape(P, NTC * CB * QW)
        )
        xT = np.ascontiguousarray(xT).astype(NPBF)
        in_maps.append(
            dict(xT=xT, wqkT=wqkT, bqk=bqk, wvT=wvT, woT=woT, cosF=cosF_b,
                 sinF=sinF_b, psw=psw_b, band=band_b)
        )

    res = run_bass_kernel_spmd(nc, in_maps, core_ids=list(range(NCORES)),
                               trace=_trace, tmpdir=_tmpdir)
    y = np.zeros((B, T, C), np.float32)
    for core in range(NCORES):
        y[core // 2] += res.results[core]["y"].astype(np.float32)
    # v-bias contribution: sum_k softmax_k (v_k + b_v) = (sum) + b_v, so
    # y gains the constant row b_v @ W_o^T; fold it in with b_o here.
    b_v = b_attn[2 * C :]
    y += (W_o @ b_v + b_o)[None, None, :]
    if _trace:
        _cache["last_result"] = res
    return y


# revision 21
# speedup vs baseline: 1.2396x; 1.0175x over previous
"""Causal self-attention (B=4, T=2048, C=1024, NH=16, HS=64) on 8 trn2 cores.

Sharding: core = (batch b, head-group hg): b = core//2, hg = core%2.
Each core computes 8 heads of one batch: column-parallel W_attn (its heads'
q/k rows), row-parallel W_o (its heads' columns).  Host sums the two
head-group partials per batch and adds b_o (+ W_o @ b_v: softmax weights sum
to 1, so the v-bias contributes a constant row -> folded into the host bias).

Device algorithm (per core, all matmuls bf16 inputs / fp32 PSUM):
  q^T,k^T = W_local @ x^T          (transposed layout [j, t])
  rope via q*cosF + (P_swap @ (q*sinF_signed))   (P_swap = const permutation)
  v      = (x @ W_v^T) directly in [t, d] layout (lhsT = x^T block), with
           ones columns interleaved per head -> no PE transposes, no v bias
  S^T[k,q] = K_rot^T.T @ Q_rot^T   (scores transposed, causal blocks only,
           diagonal blocks restricted to columns >= off: no wasted PE cols)
  E = exp(S^T/8) (ScalarE, fused 1/8 scale), staircase mask on diagonal
  O^T|sums = [V|1]^T.T @ E         (fused unnormalized output + denominator,
           also column-restricted on diagonal blocks)
  O_norm = O^T * (1/sums)          (DVE recip, TensorE rank-1 broadcast)
  y = O_cat^T.T @ W_o_cols^T       (accumulate over head pairs in PSUM),
           written back as bf16 (host accumulates partials in fp32)

Perf structure (v3):
  - Two heads of a pair on partition halves 0-63/64-127: QK^T score matmuls
    run on concurrent PE row tiles; one fused [128, 2*512] exp per job.
  - Diagonal-block scores/AV matmuls only cover columns [off:]; the
    fully-masked region is never computed (saves ~37k PE cycles/core) and
    the es zero-memsets disappear.
  - V^T is computed straight from the projection (lhsT = x^T 128-block), so
    the 128 per-pair PE transposes + v bias adds of v2 are gone.
  - First weight/x DMA pieces are cb-pair sized so the first matmul starts
    ~5us in; bias rides first on the gpsimd queue; cos/sin land before the
    first rope chunk; wo on the scalar queue.
  - y stored bf16 (halves writeback bytes; host sums partials in fp32).

Head-dim channels are reordered on the host (per head: even dims then odd
dims) so RoPE pairs live in contiguous 32-partition blocks; attention scores
are invariant to this permutation since q and k use the same order, and v/W_o
stay in natural order.
"""

from contextlib import ExitStack
from itertools import chain

import numpy as np
import ml_dtypes

import concourse.bass as bass
import concourse.mybir as mybir
import concourse.tile as tile
from concourse.bass_utils import run_bass_kernel_spmd
from concourse.masks import make_identity

B, T, C = 4, 2048, 1024
NH, HS = 16, 64
P = 128
NCORES = 8
NPAIR = 4            # head pairs per core (8 local heads)
CB = C // P          # 8 contraction blocks over C
QW = 512             # q-chunk width
NTC = T // QW        # 4 q-chunks
NKB = T // P         # 16 key blocks
F32 = mybir.dt.float32
BF16 = mybir.dt.bfloat16
NPBF = ml_dtypes.bfloat16
AF = mybir.ActivationFunctionType
ALU = mybir.AluOpType

_cache = {}


def _legalize_waits(nc, max_waits=1):
    """The walrus build here allows only one sync-wait command per
    instruction; move excess Tile-generated waits onto preceding
    single-wait NoOps on the same engine (same-engine program order
    makes this equivalent)."""
    n_id = [0]
    for fn in nc.m.functions:
        for blk in fn.blocks:
            out = []
            for inst in blk.instructions:
                si = inst.sync_info
                if si is not None and si.on_wait and len(si.on_wait) > max_waits:
                    waits = list(si.on_wait)
                    excess, keep = waits[:-max_waits], waits[-max_waits:]
                    for w in excess:
                        n_id[0] += 1
                        out.append(
                            mybir.InstNoOp(
                                name=f"waitsplit-{n_id[0]}",
                                engine=inst.engine,
                                bass_nofuse=True,
                                sync_info=mybir.SyncInfo(
                                    on_wait=[w], on_update=[]
                                ),
                            )
                        )
                    inst.sync_info = mybir.SyncInfo(
                        on_wait=keep, on_update=list(si.on_update)
                    )
                out.append(inst)
            blk.instructions = out
    return nc


def _build_nc():
    nc = bass.Bass(target_bir_lowering=True)
    # all large inputs are pre-swizzled on the host so every DMA reads
    # contiguous multi-KB runs per partition (small-packet strided DMAs
    # run at a fraction of queue bandwidth)
    xT_d = nc.dram_tensor("xT", [P, NTC * CB * QW], BF16, kind="ExternalInput")
    w_d = nc.dram_tensor("wqkT", [P, NPAIR * CB * 2 * P], BF16,
                         kind="ExternalInput")
    b_d = nc.dram_tensor("bqk", [P, 8], F32, kind="ExternalInput")
    wv_d = nc.dram_tensor("wvT", [P, CB * 4 * P], BF16, kind="ExternalInput")
    wo_d = nc.dram_tensor("woT", [P, NPAIR * C], BF16, kind="ExternalInput")
    cos_d = nc.dram_tensor("cosF", [P, T], BF16, kind="ExternalInput")
    sin_d = nc.dram_tensor("sinF", [P, T], BF16, kind="ExternalInput")
    psw_d = nc.dram_tensor("psw", [P, P], BF16, kind="ExternalInput")
    band_d = nc.dram_tensor("band", [P, 2 * P], BF16, kind="ExternalInput")
    y_d = nc.dram_tensor("y", [T, C], BF16, kind="ExternalOutput")

    with tile.TileContext(nc) as tc, ExitStack() as ctx:
        const = ctx.enter_context(tc.tile_pool(name="const", bufs=1))
        wpool = ctx.enter_context(tc.tile_pool(name="wpool", bufs=2))
        qkpool = ctx.enter_context(tc.tile_pool(name="qkpool", bufs=4))
        tmppool = ctx.enter_context(tc.tile_pool(name="tmppool", bufs=2))
        rotpool = ctx.enter_context(tc.tile_pool(name="rotpool", bufs=4))
        epool = ctx.enter_context(tc.tile_pool(name="epool", bufs=8))
        extpool = ctx.enter_context(tc.tile_pool(name="extpool", bufs=3))
        rcppool = ctx.enter_context(tc.tile_pool(name="rcppool", bufs=2))
        opool = ctx.enter_context(tc.tile_pool(name="opool", bufs=4))
        ypool = ctx.enter_context(tc.tile_pool(name="ypool", bufs=2))
        # PSUM budget (8 banks): qk 2x[P,2,QW]=4, av 2x[65,QW]=2, misc 2x[P,QW]=2
        ps_qk = ctx.enter_context(tc.tile_pool(name="ps_qk", bufs=2, space="PSUM"))
        ps_av = ctx.enter_context(tc.tile_pool(name="ps_av", bufs=2, space="PSUM"))
        ps_m = ctx.enter_context(tc.tile_pool(name="ps_m", bufs=2, space="PSUM"))

        # ---- loads, ordered so pair-0 compute can start within ~5us ----
        def load_w(w_sb, p, splits=(8,)):
            base = p * CB * 2 * P
            cb0 = 0
            for npc in splits:
                sl = slice(base + cb0 * 2 * P, base + (cb0 + npc) * 2 * P)
                nc.sync.dma_start(
                    w_sb[:, cb0 : cb0 + npc, :],
                    w_d[:, sl].rearrange("p (cb j) -> p cb j", cb=npc),
                )
                cb0 += npc

        w0_sb = wpool.tile([P, CB, 2 * P], BF16, tag="w", name="w_sb")
        load_w(w0_sb, 0, splits=(1, 1, 2, 4))

        xT_sb = []
        for tc_i in range(NTC):
            xT_sb.append(const.tile([P, CB, QW], BF16, name=f"xT{tc_i}",
                                    tag=f"xT{tc_i}"))

        def load_x(tc_i, eng, splits=(4, 4)):
            xt = xT_sb[tc_i]
            base = tc_i * CB * QW
            cb0 = 0
            for npc in splits:
                sl = slice(base + cb0 * QW, base + (cb0 + npc) * QW)
                eng.dma_start(
                    xt[:, cb0 : cb0 + npc, :],
                    xT_d[:, sl].rearrange("p (cb q) -> p cb q", cb=npc),
                )
                cb0 += npc

        # scalar queue: xT0 (fine pieces), xT2, wo (needed late)
        load_x(0, nc.scalar, splits=(1, 1, 2, 4))
        # gpsimd queue: bias (tiny, needed by first proj output), xT1,
        # psw+cos+sin (first rope chunk ~18us), xT3, band (first diag exp)
        bias_sb = const.tile([P, 8], F32)
        nc.gpsimd.dma_start(bias_sb[:], b_d[:])
        load_x(1, nc.gpsimd, splits=(2, 2, 4))
        load_x(2, nc.scalar, splits=(4, 4))
        psw_sb = const.tile([P, P], BF16)
        nc.gpsimd.dma_start(psw_sb[:], psw_d[:])
        cos_sb = const.tile([P, T], BF16)
        sin_sb = const.tile([P, T], BF16)
        nc.gpsimd.dma_start(cos_sb[:, : T // 2], cos_d[:, : T // 2])
        nc.gpsimd.dma_start(sin_sb[:, : T // 2], sin_d[:, : T // 2])
        # sync queue (after pair-0 qk weights): xT3, then v weights for the
        # direct v^T projection (first vt block needs all cb of wv)
        load_x(3, nc.sync, splits=(4, 4))
        wv_sb = const.tile([P, CB, 4 * P], BF16)
        for i in range(2):
            sl = slice(i * 4 * 4 * P, (i + 1) * 4 * 4 * P)
            nc.sync.dma_start(
                wv_sb[:, i * 4 : (i + 1) * 4, :],
                wv_d[:, sl].rearrange("p (cb j) -> p cb j", cb=4),
            )
        nc.gpsimd.dma_start(cos_sb[:, T // 2 :], cos_d[:, T // 2 :])
        nc.gpsimd.dma_start(sin_sb[:, T // 2 :], sin_d[:, T // 2 :])
        band_sb = const.tile([P, 2, P], BF16)
        nc.gpsimd.dma_start(band_sb[:],
                            band_d.rearrange("p (g q) -> p g q", g=2))
        wo_sb = const.tile([P, NPAIR, C], BF16)
        nc.scalar.dma_start(wo_sb[:], wo_d.rearrange("p (pr o) -> p pr o",
                                                     pr=NPAIR))
        ident = const.tile([P, P], BF16)
        make_identity(nc, ident[:])
        ones64 = const.tile([65, HS], BF16)
        nc.gpsimd.memset(ones64[64:65, :], 1.0)

        # v in [t, d] layout: vn[:, kb, p, h, 0:64] = v dims, [..., 64] = 1.0
        # (interleaved ones columns keep each head's [P, 65] lhsT contiguous)
        vn = const.tile([P, NKB, NPAIR, 2, HS + 1], BF16, name="vn")
        nc.gpsimd.memset(vn[:, :, :, :, HS : HS + 1], 1.0)

        ocat = [opool.tile([P, T], BF16, name=f"ocat{p}", tag="ocat")
                for p in range(NPAIR)]

        prep_out = {}

        def prep_stream(p, w_sb):
            """q/k projection + rope for pair p, chunk-major (each x chunk
            is fully consumed before the next is touched, which keeps early
            PE demand under the DMA feed rate).  Yields between PE-sized
            chunks; emission order sets scheduler priority so this work
            fills pair p-1's attention exp-wait bubbles."""
            qk = [qkpool.tile([P, T], BF16, tag="qkT", name="qkT")
                  for _ in range(2)]
            rots = [rotpool.tile([P, T], BF16, tag="rot", name="rot")
                    for _ in range(2)]
            for tc_i in range(NTC):
                sl = slice(tc_i * QW, (tc_i + 1) * QW)
                for jb in range(2):
                    dst = qk[jb]
                    bias_bc = bias_sb[
                        :, 2 * p + jb : 2 * p + jb + 1
                    ].to_broadcast((P, QW))
                    psum = ps_m.tile([P, QW], F32, tag="m", name="pj")
                    for cb in range(CB):
                        nc.tensor.matmul(
                            psum[:],
                            lhsT=w_sb[:, cb, jb * P : (jb + 1) * P],
                            rhs=xT_sb[tc_i][:, cb, :],
                            start=(cb == 0),
                            stop=(cb == CB - 1),
                        )
                    nc.vector.tensor_tensor(
                        dst[:, sl], psum[:], bias_bc, ALU.add,
                    )
                    yield
                for jb in range(2):  # rot = t*cos + P_swap @ (t*sin_signed)
                    src, rot = qk[jb], rots[jb]
                    sq = tmppool.tile([P, QW], BF16, tag="sq", name="sq")
                    nc.vector.tensor_tensor(
                        sq[:], src[:, sl], sin_sb[:, sl], ALU.mult
                    )
                    nc.vector.tensor_tensor(
                        rot[:, sl], src[:, sl], cos_sb[:, sl], ALU.mult
                    )
                    psum = ps_m.tile([P, QW], F32, tag="m", name="sw")
                    nc.tensor.matmul(
                        psum[:],
                        lhsT=psw_sb[:],
                        rhs=sq[:],
                        start=True,
                        stop=True,
                    )
                    nc.vector.tensor_tensor(
                        rot[:, sl], rot[:, sl], psum[:], ALU.add,
                    )
                    yield
            prep_out[p] = (rots[0], rots[1])

        def vt_stream(tb_lo, tb_hi):
            """v^T for 128-row blocks tb_lo..tb_hi-1, all pairs at once:
            psum[t, (p,h,d)] = sum_c x^T[c, t-block]^T wv^T[c, (p,h,d)]."""
            for tb in range(tb_lo, tb_hi):
                tci, loc = tb // 4, tb % 4
                psum = ps_m.tile([P, QW], F32, tag="m", name="vt")
                for cb in range(CB):
                    nc.tensor.matmul(
                        psum[:],
                        lhsT=xT_sb[tci][:, cb, loc * P : (loc + 1) * P],
                        rhs=wv_sb[:, cb, :],
                        start=(cb == 0),
                        stop=(cb == CB - 1),
                    )
                nc.vector.tensor_copy(
                    vn[:, tb, :, :, :HS],
                    psum[:].rearrange("p (pr h d) -> p pr h d", pr=NPAIR, h=2),
                )
                yield

        def emit_outproj_block(tb):
            """y[tb*P:(tb+1)*P, :] = sum_p ocat_p^T @ woT_p for one 128-row
            block; emitted as soon as all pairs' ocat columns are final."""
            for oc in range(2):
                psum = ps_m.tile([P, QW], F32, tag="m", name="yp")
                for p in range(NPAIR):
                    nc.tensor.matmul(
                        psum[:],
                        lhsT=ocat[p][:, tb * P : (tb + 1) * P],
                        rhs=wo_sb[:, p, oc * QW : (oc + 1) * QW],
                        start=(p == 0),
                        stop=(p == NPAIR - 1),
                    )
                yb = ypool.tile([P, QW], BF16, tag="yb")
                nc.vector.tensor_copy(yb[:], psum[:])
                eng = nc.sync if (2 * tb + oc) % 2 == 0 else nc.gpsimd
                eng.dma_start(
                    y_d[tb * P : (tb + 1) * P, oc * QW : (oc + 1) * QW],
                    yb[:],
                )

        def attn_stream(p):
            """Attention for pair p.  Jobs are packed per (qc, kb): both
            heads' QK^T matmuls are emitted back-to-back (concurrent PE row
            tiles 0-63 / 64-127), followed by one fused exp over both heads'
            PSUM banks and the two AV accumulations.  Diagonal blocks only
            compute columns [off:]."""
            rq, rk = prep_out[p]
            ps_o_cur = {}

            def normalize(ext2, qc, heads):
                # 1/sums: ln+exp on the partition-64 sums row (same ACT
                # table as the scores exp -> no table reload)
                rcpf = rcppool.tile([65, 2, QW], F32, tag="rcpf", name="rcpf")
                nc.scalar.activation(
                    rcpf[64:65, heads, :], ext2[64:65, heads, :], AF.Ln
                )
                rcpb = rcppool.tile([65, 2, QW], BF16, tag="rcpb", name="rcpb")
                nc.scalar.activation(
                    rcpb[64:65, heads, :], rcpf[64:65, heads, :],
                    AF.Exp, scale=-1.0,
                )
                hs = range(2) if heads == slice(None) else [heads]
                for h in hs:
                    rb = ps_m.tile([HS, QW], F32, tag="m", name="rb")
                    nc.tensor.matmul(
                        rb[:],
                        lhsT=ones64[64:65, :],
                        rhs=rcpb[64:65, h, :],
                        start=True,
                        stop=True,
                    )
                    nc.vector.tensor_tensor(
                        ocat[p][h * HS : (h + 1) * HS,
                                qc * QW : (qc + 1) * QW],
                        ext2[:HS, h, :],
                        rb[:],
                        ALU.mult,
                    )

            for qc in range(NTC):
                nkb = 4 * (qc + 1)
                for h in range(2):
                    ps_o_cur[h] = ps_av.tile([HS + 1, QW], F32, tag="av",
                                             name="ps_o")
                for kb in range(nkb):
                    off = max(0, P * (kb - 4 * qc))
                    ps_s = ps_qk.tile([P, 2, QW], F32, tag="qk", name="ps_s")
                    for h in range(2):
                        nc.tensor.matmul(
                            ps_s[:, h, off:],
                            lhsT=rk[h * HS : (h + 1) * HS,
                                    kb * P : (kb + 1) * P],
                            rhs=rq[h * HS : (h + 1) * HS,
                                   qc * QW + off : (qc + 1) * QW],
                            start=True,
                            stop=True,
                        )
                    es = epool.tile([P, 2, QW], BF16, tag="es", name="es")
                    if off == 0:
                        nc.scalar.activation(
                            es[:].rearrange("p g q -> p (g q)"),
                            ps_s[:].rearrange("p g q -> p (g q)"),
                            AF.Exp,
                            scale=0.125,
                        )
                    else:  # diagonal block: partial exp + staircase mask
                        nc.scalar.activation(
                            es[:, :, off:], ps_s[:, :, off:],
                            AF.Exp, scale=0.125,
                        )
                    if kb - 4 * qc >= 0:
                        nc.vector.tensor_tensor(
                            es[:, :, off : off + P],
                            es[:, :, off : off + P],
                            band_sb[:],
                            ALU.mult,
                        )
                    # Late priority: when several PE instructions are ready
                    # the scheduler prefers QKT/prep work, so AVs trail exp
                    # by as much as the es pool depth allows (robust to
                    # cost-model vs hardware timing skew).
                    with tc.high_priority(offset=-1_000_000):
                        for h in range(2):
                            nc.tensor.matmul(
                                ps_o_cur[h][:, off:],
                                lhsT=vn[:, kb, p, h, :],
                                rhs=es[:, h, off:],
                                start=(kb == 0),
                                stop=(kb == nkb - 1),
                            )
                    yield
                # pair 3's normalize feeds the out-projection: keep it eager
                # and per-head there (shortest latency to the first rank-1);
                # defer it elsewhere (nothing reads ocat until pair 3)
                if p < NPAIR - 1:
                    with tc.high_priority(offset=-1_000_000):
                        ext2 = extpool.tile([HS + 1, 2, QW], F32, tag="ext",
                                            name="ext2")
                        for h in range(2):
                            nc.vector.tensor_copy(ext2[:, h, :],
                                                  ps_o_cur[h][:])
                        normalize(ext2, qc, slice(None))
                else:
                    ext2 = extpool.tile([HS + 1, 2, QW], F32, tag="ext",
                                        name="ext2")
                    for h in range(2):
                        nc.vector.tensor_copy(ext2[:, h, :], ps_o_cur[h][:])
                        normalize(ext2, qc, h)
                yield
                if p == NPAIR - 1:
                    with tc.high_priority(offset=-1_000_000):
                        for tb in range(4 * qc, 4 * qc + 4):
                            emit_outproj_block(tb)
                    yield

        def drive(a_gen, b_gen, ratio=2):
            done_a = a_gen is None
            done_b = b_gen is None
            while not (done_a and done_b):
                if not done_a:
                    for _ in range(ratio):
                        try:
                            next(a_gen)
                        except StopIteration:
                            done_a = True
                            break
                if not done_b:
                    try:
                        next(b_gen)
                    except StopIteration:
                        done_b = True

        w_tiles = {0: w0_sb}
        w_tiles[1] = wpool.tile([P, CB, 2 * P], BF16, tag="w", name="w_sb")
        load_w(w_tiles[1], 1)
        for _ in prep_stream(0, w0_sb):
            pass
        for _ in vt_stream(0, 4):
            pass
        for p in range(NPAIR):
            if p + 1 < NPAIR:
                if p + 2 < NPAIR:
                    w_tiles[p + 2] = wpool.tile([P, CB, 2 * P], BF16,
                                                tag="w", name="w_sb")
                    load_w(w_tiles[p + 2], p + 2)
                companion = prep_stream(p + 1, w_tiles[p + 1])
                if p == 0:
                    companion = chain(vt_stream(4, NKB), companion)
            else:
                companion = None
            drive(attn_stream(p), companion, ratio=2)
    return _legalize_waits(nc)


def _rope_tables():
    inv = 1.0 / (1000.0 ** (np.arange(0, HS, 2, dtype=np.float64) / HS))
    t = np.arange(T, dtype=np.float64)[:, None] * inv[None, :]
    sinT = np.sin(t).astype(np.float32).T  # [32, T]
    cosT = np.cos(t).astype(np.float32).T
    cosF = np.concatenate([cosT] * 4, 0)  # [128, T]
    # sign layout for multiply-BEFORE-swap: sq = q*sinF, swapped(sq) lands as
    # [-v*sin; +u*sin] in the [u; v] destination slots.
    sinF = np.concatenate([sinT, -sinT, sinT, -sinT], 0)
    return cosF, sinF


def _host_prep():
    cosF, sinF = _rope_tables()
    psw = np.zeros((P, P), np.float32)
    for hh in range(2):
        o = hh * HS
        psw[o : o + 32, o + 32 : o + 64] = np.eye(32)
        psw[o + 32 : o + 64, o : o + 32] = np.eye(32)
    # band[p, j] = 1 iff j >= p: causal triangle for the 128-wide diagonal
    # band, replicated for the two packed heads
    tri = np.tril(np.ones((P, P), np.float32)).T
    band = np.concatenate([tri, tri], axis=1)  # [P, 2P]
    return cosF, sinF, psw, band


def kernel(x, W_attn, b_attn, W_o, b_o, _trace=False, _tmpdir=None):
    x = np.asarray(x, np.float32)
    W_attn = np.asarray(W_attn, np.float32)
    b_attn = np.asarray(b_attn, np.float32)
    W_o = np.asarray(W_o, np.float32)
    b_o = np.asarray(b_o, np.float32)

    if "nc" not in _cache:
        _cache["nc"] = _build_nc()
    nc = _cache["nc"]

    cosF, sinF, psw, band = _host_prep()
    cosF_b, sinF_b = cosF.astype(NPBF), sinF.astype(NPBF)
    psw_b, band_b = psw.astype(NPBF), band.astype(NPBF)

    def head_rows(h):  # q-rows of head h, evens then odds
        base = h * HS
        return np.concatenate(
            [np.arange(base, base + HS, 2), np.arange(base + 1, base + HS, 2)]
        )

    in_maps = []
    for core in range(NCORES):
        b, hg = core // 2, core % 2
        heads = [hg * 8 + i for i in range(8)]
        qk_rows = []
        v_rows = []
        for p in range(NPAIR):
            h0, h1 = heads[2 * p], heads[2 * p + 1]
            qrows = np.concatenate([head_rows(h0), head_rows(h1)])
            qk_rows += [qrows, C + qrows]
            v_rows += [2 * C + np.concatenate(
                [np.arange(h0 * HS, (h0 + 1) * HS),
                 np.arange(h1 * HS, (h1 + 1) * HS)])]
        qk_rows = np.concatenate(qk_rows)  # [1024] pair-major (q,k) order
        v_rows = np.concatenate(v_rows)    # [512]  pair-major natural order
        # device-friendly swizzles: partition-major with contiguous per-
        # partition runs ([P, ...]) so DMAs move multi-KB packets
        wqkT = (
            W_attn[qk_rows].T.reshape(CB, P, NPAIR, 2 * P)
            .transpose(1, 2, 0, 3)
            .reshape(P, NPAIR * CB * 2 * P)
        )
        wqkT = np.ascontiguousarray(wqkT).astype(NPBF)
        bqk = np.ascontiguousarray(b_attn[qk_rows].reshape(8, P).T)  # [P, 8]
        wvT = (
            W_attn[v_rows].T.reshape(CB, P, 4 * P)
            .transpose(1, 0, 2)
            .reshape(P, CB * 4 * P)
        )
        wvT = np.ascontiguousarray(wvT).astype(NPBF)
        woT = (
            W_o[:, hg * 512 : (hg + 1) * 512].T.reshape(NPAIR, P, C)
            .transpose(1, 0, 2)
            .reshape(P, NPAIR * C)
        )
        woT = np.ascontiguousarray(woT).astype(NPBF)
        xT = (
            x[b].T.reshape(CB, P, NTC, QW)
            .transpose(1, 2, 0, 3)
            .reshape(P, NTC * CB * QW)
        )
        xT = np.ascontiguousarray(xT).astype(NPBF)
        in_maps.append(
            dict(xT=xT, wqkT=wqkT, bqk=bqk, wvT=wvT, woT=woT, cosF=cosF_b,
                 sinF=sinF_b, psw=psw_b, band=band_b)
        )

    res = run_bass_kernel_spmd(nc, in_maps, core_ids=list(range(NCORES)),
                               trace=_trace, tmpdir=_tmpdir)
    y = np.zeros((B, T, C), np.float32)
    for core in range(NCORES):
        y[core // 2] += res.results[core]["y"].astype(np.float32)
    # v-bias contribution: sum_k softmax_k (v_k + b_v) = (sum) + b_v, so
    # y gains the constant row b_v @ W_o^T; fold it in with b_o here.
    b_v = b_attn[2 * C :]
    y += (W_o @ b_v + b_o)[None, None, :]
    if _trace:
        _cache["last_result"] = res
    return y


# revision 24
# speedup vs baseline: 1.2414x; 1.0015x over previous
"""Causal self-attention (B=4, T=2048, C=1024, NH=16, HS=64) on 8 trn2 cores.

Sharding: core = (batch b, head-group hg): b = core//2, hg = core%2.
Each core computes 8 heads of one batch: column-parallel W_attn (its heads'
q/k rows), row-parallel W_o (its heads' columns).  Host sums the two
head-group partials per batch and adds b_o (+ W_o @ b_v: softmax weights sum
to 1, so the v-bias contributes a constant row -> folded into the host bias).

Device algorithm (per core, all matmuls bf16 inputs / fp32 PSUM):
  q^T,k^T = W_local @ x^T          (transposed layout [j, t])
  rope via q*cosF + (P_swap @ (q*sinF_signed))   (P_swap = const permutation)
  v      = (x @ W_v^T) directly in [t, d] layout (lhsT = x^T block), with
           ones columns interleaved per head -> no PE transposes, no v bias
  S^T[k,q] = K_rot^T.T @ Q_rot^T   (scores transposed, causal blocks only,
           diagonal blocks restricted to columns >= off: no wasted PE cols)
  E = exp(S^T/8) (ScalarE, fused 1/8 scale), staircase mask on diagonal
  O^T|sums = [V|1]^T.T @ E         (fused unnormalized output + denominator,
           also column-restricted on diagonal blocks)
  O_norm = O^T * (1/sums)          (DVE recip, TensorE rank-1 broadcast)
  y = O_cat^T.T @ W_o_cols^T       (accumulate over head pairs in PSUM),
           written back as bf16 (host accumulates partials in fp32)

Perf structure (v3):
  - Two heads of a pair on partition halves 0-63/64-127: QK^T score matmuls
    run on concurrent PE row tiles; one fused [128, 2*512] exp per job.
  - Diagonal-block scores/AV matmuls only cover columns [off:]; the
    fully-masked region is never computed (saves ~37k PE cycles/core) and
    the es zero-memsets disappear.
  - V^T is computed straight from the projection (lhsT = x^T 128-block), so
    the 128 per-pair PE transposes + v bias adds of v2 are gone.
  - First weight/x DMA pieces are cb-pair sized so the first matmul starts
    ~5us in; bias rides first on the gpsimd queue; cos/sin land before the
    first rope chunk; wo on the scalar queue.
  - y stored bf16 (halves writeback bytes; host sums partials in fp32).

Head-dim channels are reordered on the host (per head: even dims then odd
dims) so RoPE pairs live in contiguous 32-partition blocks; attention scores
are invariant to this permutation since q and k use the same order, and v/W_o
stay in natural order.
"""

from contextlib import ExitStack
from itertools import chain

import numpy as np
import ml_dtypes

import concourse.bass as bass
import concourse.mybir as mybir
import concourse.tile as tile
from concourse.bass_utils import run_bass_kernel_spmd
from concourse.masks import make_identity

B, T, C = 4, 2048, 1024
NH, HS = 16, 64
P = 128
NCORES = 8
NPAIR = 4            # head pairs per core (8 local heads)
CB = C // P          # 8 contraction blocks over C
QW = 512             # q-chunk width
NTC = T // QW        # 4 q-chunks
NKB = T // P         # 16 key blocks
F32 = mybir.dt.float32
BF16 = mybir.dt.bfloat16
NPBF = ml_dtypes.bfloat16
AF = mybir.ActivationFunctionType
ALU = mybir.AluOpType

_cache = {}


def _legalize_waits(nc, max_waits=1):
    """The walrus build here allows only one sync-wait command per
    instruction; move excess Tile-generated waits onto preceding
    single-wait NoOps on the same engine (same-engine program order
    makes this equivalent)."""
    n_id = [0]
    for fn in nc.m.functions:
        for blk in fn.blocks:
            out = []
            for inst in blk.instructions:
                si = inst.sync_info
                if si is not None and si.on_wait and len(si.on_wait) > max_waits:
                    waits = list(si.on_wait)
                    excess, keep = waits[:-max_waits], waits[-max_waits:]
                    for w in excess:
                        n_id[0] += 1
                        out.append(
                            mybir.InstNoOp(
                                name=f"waitsplit-{n_id[0]}",
                                engine=inst.engine,
                                bass_nofuse=True,
                                sync_info=mybir.SyncInfo(
                                    on_wait=[w], on_update=[]
                                ),
                            )
                        )
                    inst.sync_info = mybir.SyncInfo(
                        on_wait=keep, on_update=list(si.on_update)
                    )
                out.append(inst)
            blk.instructions = out
    return nc


def _build_nc():
    nc = bass.Bass(target_bir_lowering=True)
    # all large inputs are pre-swizzled on the host so every DMA reads
    # contiguous multi-KB runs per partition (small-packet strided DMAs
    # run at a fraction of queue bandwidth)
    xT_d = nc.dram_tensor("xT", [P, NTC * CB * QW], BF16, kind="ExternalInput")
    w_d = nc.dram_tensor("wqkT", [P, NPAIR * CB * 2 * P], BF16,
                         kind="ExternalInput")
    b_d = nc.dram_tensor("bqk", [P, 8], F32, kind="ExternalInput")
    wv_d = nc.dram_tensor("wvT", [P, CB * 4 * P], BF16, kind="ExternalInput")
    wo_d = nc.dram_tensor("woT", [P, NPAIR * C], BF16, kind="ExternalInput")
    cos_d = nc.dram_tensor("cosF", [P, T], BF16, kind="ExternalInput")
    sin_d = nc.dram_tensor("sinF", [P, T], BF16, kind="ExternalInput")
    psw_d = nc.dram_tensor("psw", [P, P], BF16, kind="ExternalInput")
    band_d = nc.dram_tensor("band", [P, 2 * P], BF16, kind="ExternalInput")
    y_d = nc.dram_tensor("y", [T, C], BF16, kind="ExternalOutput")

    with tile.TileContext(nc) as tc, ExitStack() as ctx:
        const = ctx.enter_context(tc.tile_pool(name="const", bufs=1))
        wpool = ctx.enter_context(tc.tile_pool(name="wpool", bufs=2))
        qkpool = ctx.enter_context(tc.tile_pool(name="qkpool", bufs=4))
        tmppool = ctx.enter_context(tc.tile_pool(name="tmppool", bufs=2))
        rotpool = ctx.enter_context(tc.tile_pool(name="rotpool", bufs=4))
        epool = ctx.enter_context(tc.tile_pool(name="epool", bufs=8))
        extpool = ctx.enter_context(tc.tile_pool(name="extpool", bufs=3))
        rcppool = ctx.enter_context(tc.tile_pool(name="rcppool", bufs=2))
        opool = ctx.enter_context(tc.tile_pool(name="opool", bufs=4))
        ypool = ctx.enter_context(tc.tile_pool(name="ypool", bufs=2))
        # PSUM budget (8 banks): qk 2x[P,2,QW]=4, av 2x[65,QW]=2, misc 2x[P,QW]=2
        ps_qk = ctx.enter_context(tc.tile_pool(name="ps_qk", bufs=2, space="PSUM"))
        ps_av = ctx.enter_context(tc.tile_pool(name="ps_av", bufs=2, space="PSUM"))
        ps_m = ctx.enter_context(tc.tile_pool(name="ps_m", bufs=2, space="PSUM"))

        # ---- loads, ordered so pair-0 compute can start within ~5us ----
        def load_w(w_sb, p, splits=(8,)):
            base = p * CB * 2 * P
            cb0 = 0
            for npc in splits:
                sl = slice(base + cb0 * 2 * P, base + (cb0 + npc) * 2 * P)
                nc.sync.dma_start(
                    w_sb[:, cb0 : cb0 + npc, :],
                    w_d[:, sl].rearrange("p (cb j) -> p cb j", cb=npc),
                )
                cb0 += npc

        w0_sb = wpool.tile([P, CB, 2 * P], BF16, tag="w", name="w_sb")
        load_w(w0_sb, 0, splits=(1, 1, 2, 4))

        xT_sb = []
        for tc_i in range(NTC):
            xT_sb.append(const.tile([P, CB, QW], BF16, name=f"xT{tc_i}",
                                    tag=f"xT{tc_i}"))

        def load_x(tc_i, eng, splits=(4, 4)):
            xt = xT_sb[tc_i]
            base = tc_i * CB * QW
            cb0 = 0
            for npc in splits:
                sl = slice(base + cb0 * QW, base + (cb0 + npc) * QW)
                eng.dma_start(
                    xt[:, cb0 : cb0 + npc, :],
                    xT_d[:, sl].rearrange("p (cb q) -> p cb q", cb=npc),
                )
                cb0 += npc

        # scalar queue: xT0 (fine pieces), xT2, wo (needed late)
        load_x(0, nc.scalar, splits=(1, 1, 2, 4))
        # gpsimd queue: bias (tiny, needed by first proj output), xT1,
        # psw+cos+sin (first rope chunk ~18us), xT3, band (first diag exp)
        bias_sb = const.tile([P, 8], F32)
        nc.gpsimd.dma_start(bias_sb[:], b_d[:])
        load_x(1, nc.gpsimd, splits=(2, 2, 4))
        load_x(2, nc.scalar, splits=(4, 4))
        psw_sb = const.tile([P, P], BF16)
        nc.gpsimd.dma_start(psw_sb[:], psw_d[:])
        cos_sb = const.tile([P, T], BF16)
        sin_sb = const.tile([P, T], BF16)
        nc.gpsimd.dma_start(cos_sb[:, : T // 2], cos_d[:, : T // 2])
        nc.gpsimd.dma_start(sin_sb[:, : T // 2], sin_d[:, : T // 2])
        # sync queue (after pair-0 qk weights): xT3, then v weights for the
        # direct v^T projection (first vt block needs all cb of wv)
        load_x(3, nc.sync, splits=(4, 4))
        wv_sb = const.tile([P, CB, 4 * P], BF16)
        for i in range(2):
            sl = slice(i * 4 * 4 * P, (i + 1) * 4 * 4 * P)
            nc.sync.dma_start(
                wv_sb[:, i * 4 : (i + 1) * 4, :],
                wv_d[:, sl].rearrange("p (cb j) -> p cb j", cb=4),
            )
        nc.gpsimd.dma_start(cos_sb[:, T // 2 :], cos_d[:, T // 2 :])
        nc.gpsimd.dma_start(sin_sb[:, T // 2 :], sin_d[:, T // 2 :])
        band_sb = const.tile([P, 2, P], BF16)
        nc.gpsimd.dma_start(band_sb[:],
                            band_d.rearrange("p (g q) -> p g q", g=2))
        wo_sb = const.tile([P, NPAIR, C], BF16)
        nc.scalar.dma_start(wo_sb[:], wo_d.rearrange("p (pr o) -> p pr o",
                                                     pr=NPAIR))
        ident = const.tile([P, P], BF16)
        make_identity(nc, ident[:])
        ones64 = const.tile([65, HS], BF16)
        nc.gpsimd.memset(ones64[64:65, :], 1.0)

        # v in [t, d] layout: vn[:, kb, p, h, 0:64] = v dims, [..., 64] = 1.0
        # (interleaved ones columns keep each head's [P, 65] lhsT contiguous)
        vn = const.tile([P, NKB, NPAIR, 2, HS + 1], BF16, name="vn")
        nc.gpsimd.memset(vn[:, :, :, :, HS : HS + 1], 1.0)

        ocat = [opool.tile([P, T], BF16, name=f"ocat{p}", tag="ocat")
                for p in range(NPAIR)]

        prep_out = {}

        def prep_stream(p, w_sb):
            """q/k projection + rope for pair p, chunk-major (each x chunk
            is fully consumed before the next is touched, which keeps early
            PE demand under the DMA feed rate).  Yields between PE-sized
            chunks; emission order sets scheduler priority so this work
            fills pair p-1's attention exp-wait bubbles."""
            qk = [qkpool.tile([P, T], BF16, tag="qkT", name="qkT")
                  for _ in range(2)]
            rots = [rotpool.tile([P, T], BF16, tag="rot", name="rot")
                    for _ in range(2)]
            for tc_i in range(NTC):
                sl = slice(tc_i * QW, (tc_i + 1) * QW)
                for jb in range(2):
                    dst = qk[jb]
                    bias_bc = bias_sb[
                        :, 2 * p + jb : 2 * p + jb + 1
                    ].to_broadcast((P, QW))
                    psum = ps_m.tile([P, QW], F32, tag="m", name="pj")
                    for cb in range(CB):
                        nc.tensor.matmul(
                            psum[:],
                            lhsT=w_sb[:, cb, jb * P : (jb + 1) * P],
                            rhs=xT_sb[tc_i][:, cb, :],
                            start=(cb == 0),
                            stop=(cb == CB - 1),
                        )
                    nc.vector.tensor_tensor(
                        dst[:, sl], psum[:], bias_bc, ALU.add,
                    )
                    yield
                for jb in range(2):  # rot = t*cos + P_swap @ (t*sin_signed)
                    src, rot = qk[jb], rots[jb]
                    sq = tmppool.tile([P, QW], BF16, tag="sq", name="sq")
                    nc.vector.tensor_tensor(
                        sq[:], src[:, sl], sin_sb[:, sl], ALU.mult
                    )
                    nc.vector.tensor_tensor(
                        rot[:, sl], src[:, sl], cos_sb[:, sl], ALU.mult
                    )
                    psum = ps_m.tile([P, QW], F32, tag="m", name="sw")
                    nc.tensor.matmul(
                        psum[:],
                        lhsT=psw_sb[:],
                        rhs=sq[:],
                        start=True,
                        stop=True,
                    )
                    nc.vector.tensor_tensor(
                        rot[:, sl], rot[:, sl], psum[:], ALU.add,
                    )
                    yield
            prep_out[p] = (rots[0], rots[1])

        def vt_stream(tb_lo, tb_hi):
            """v^T for 128-row blocks tb_lo..tb_hi-1, all pairs at once:
            psum[t, (p,h,d)] = sum_c x^T[c, t-block]^T wv^T[c, (p,h,d)]."""
            for tb in range(tb_lo, tb_hi):
                tci, loc = tb // 4, tb % 4
                psum = ps_m.tile([P, QW], F32, tag="m", name="vt")
                for cb in range(CB):
                    nc.tensor.matmul(
                        psum[:],
                        lhsT=xT_sb[tci][:, cb, loc * P : (loc + 1) * P],
                        rhs=wv_sb[:, cb, :],
                        start=(cb == 0),
                        stop=(cb == CB - 1),
                    )
                nc.vector.tensor_copy(
                    vn[:, tb, :, :, :HS],
                    psum[:].rearrange("p (pr h d) -> p pr h d", pr=NPAIR, h=2),
                )
                yield

        def emit_outproj_block(tb):
            """y[tb*P:(tb+1)*P, :] = sum_p ocat_p^T @ woT_p for one 128-row
            block; emitted as soon as all pairs' ocat columns are final."""
            for oc in range(2):
                psum = ps_m.tile([P, QW], F32, tag="m", name="yp")
                for p in range(NPAIR):
                    nc.tensor.matmul(
                        psum[:],
                        lhsT=ocat[p][:, tb * P : (tb + 1) * P],
                        rhs=wo_sb[:, p, oc * QW : (oc + 1) * QW],
                        start=(p == 0),
                        stop=(p == NPAIR - 1),
                    )
                yb = ypool.tile([P, QW], BF16, tag="yb")
                nc.vector.tensor_copy(yb[:], psum[:])
                eng = nc.sync if (2 * tb + oc) % 2 == 0 else nc.gpsimd
                eng.dma_start(
                    y_d[tb * P : (tb + 1) * P, oc * QW : (oc + 1) * QW],
                    yb[:],
                )

        def attn_stream(p):
            """Attention for pair p.  Jobs are packed per (qc, kb): both
            heads' QK^T matmuls are emitted back-to-back (concurrent PE row
            tiles 0-63 / 64-127), followed by one fused exp over both heads'
            PSUM banks and the two AV accumulations.  Diagonal blocks only
            compute columns [off:]."""
            rq, rk = prep_out[p]
            ps_o_cur = {}

            def normalize(ext2, qc, heads):
                # 1/sums: ln+exp on the partition-64 sums row (same ACT
                # table as the scores exp -> no table reload)
                rcpf = rcppool.tile([65, 2, QW], F32, tag="rcpf", name="rcpf")
                nc.scalar.activation(
                    rcpf[64:65, heads, :], ext2[64:65, heads, :], AF.Ln
                )
                rcpb = rcppool.tile([65, 2, QW], BF16, tag="rcpb", name="rcpb")
                nc.scalar.activation(
                    rcpb[64:65, heads, :], rcpf[64:65, heads, :],
                    AF.Exp, scale=-1.0,
                )
                hs = range(2) if heads == slice(None) else [heads]
                for h in hs:
                    rb = ps_m.tile([HS, QW], F32, tag="m", name="rb")
                    nc.tensor.matmul(
                        rb[:],
                        lhsT=ones64[64:65, :],
                        rhs=rcpb[64:65, h, :],
                        start=True,
                        stop=True,
                    )
                    nc.vector.tensor_tensor(
                        ocat[p][h * HS : (h + 1) * HS,
                                qc * QW : (qc + 1) * QW],
                        ext2[:HS, h, :],
                        rb[:],
                        ALU.mult,
                    )

            for qc in range(NTC):
                nkb = 4 * (qc + 1)
                for h in range(2):
                    ps_o_cur[h] = ps_av.tile([HS + 1, QW], F32, tag="av",
                                             name="ps_o")
                for kb in range(nkb):
                    off = max(0, P * (kb - 4 * qc))
                    ps_s = ps_qk.tile([P, 2, QW], F32, tag="qk", name="ps_s")
                    for h in range(2):
                        nc.tensor.matmul(
                            ps_s[:, h, off:],
                            lhsT=rk[h * HS : (h + 1) * HS,
                                    kb * P : (kb + 1) * P],
                            rhs=rq[h * HS : (h + 1) * HS,
                                   qc * QW + off : (qc + 1) * QW],
                            start=True,
                            stop=True,
                        )
                    es = epool.tile([P, 2, QW], BF16, tag="es", name="es")
                    if off == 0:
                        nc.scalar.activation(
                            es[:].rearrange("p g q -> p (g q)"),
                            ps_s[:].rearrange("p g q -> p (g q)"),
                            AF.Exp,
                            scale=0.125,
                        )
                    else:  # diagonal block: partial exp + staircase mask
                        nc.scalar.activation(
                            es[:, :, off:], ps_s[:, :, off:],
                            AF.Exp, scale=0.125,
                        )
                    if kb - 4 * qc >= 0:
                        nc.vector.tensor_tensor(
                            es[:, :, off : off + P],
                            es[:, :, off : off + P],
                            band_sb[:],
                            ALU.mult,
                        )
                    # Late priority: when several PE instructions are ready
                    # the scheduler prefers QKT/prep work, so AVs trail exp
                    # by as much as the es pool depth allows (robust to
                    # cost-model vs hardware timing skew).
                    with tc.high_priority(offset=-1_000_000):
                        for h in range(2):
                            nc.tensor.matmul(
                                ps_o_cur[h][:, off:],
                                lhsT=vn[:, kb, p, h, :],
                                rhs=es[:, h, off:],
                                start=(kb == 0),
                                stop=(kb == nkb - 1),
                            )
                    yield
                # pair 3's normalize feeds the out-projection: keep it eager
                # and per-head there (shortest latency to the first rank-1);
                # defer it elsewhere (nothing reads ocat until pair 3)
                if p < NPAIR - 1:
                    # the copies free the ps_av banks for the next qc: keep
                    # them at normal priority; only the ln/exp/rank-1/mult
                    # (read ocat no earlier than pair 3) are deferred
                    ext2 = extpool.tile([HS + 1, 2, QW], F32, tag="ext",
                                        name="ext2")
                    for h in range(2):
                        nc.vector.tensor_copy(ext2[:, h, :],
                                              ps_o_cur[h][:])
                    with tc.high_priority(offset=-1_000_000):
                        normalize(ext2, qc, slice(None))
                else:
                    ext2 = extpool.tile([HS + 1, 2, QW], F32, tag="ext",
                                        name="ext2")
                    for h in range(2):
                        nc.vector.tensor_copy(ext2[:, h, :], ps_o_cur[h][:])
                        normalize(ext2, qc, h)
                yield
                if p == NPAIR - 1:
                    with tc.high_priority(offset=-1_000_000):
                        for tb in range(4 * qc, 4 * qc + 4):
                            emit_outproj_block(tb)
                    yield

        def drive(a_gen, b_gen, ratio=2):
            done_a = a_gen is None
            done_b = b_gen is None
            while not (done_a and done_b):
                if not done_a:
                    for _ in range(ratio):
                        try:
                            next(a_gen)
                        except StopIteration:
                            done_a = True
                            break
                if not done_b:
                    try:
                        next(b_gen)
                    except StopIteration:
                        done_b = True

        w_tiles = {0: w0_sb}
        w_tiles[1] = wpool.tile([P, CB, 2 * P], BF16, tag="w", name="w_sb")
        load_w(w_tiles[1], 1)
        for _ in prep_stream(0, w0_sb):
            pass
        for _ in vt_stream(0, 4):
            pass
        for p in range(NPAIR):
            if p + 1 < NPAIR:
                if p + 2 < NPAIR:
                    w_tiles[p + 2] = wpool.tile([P, CB, 2 * P], BF16,
                                                tag="w", name="w_sb")
                    load_w(w_tiles[p + 2], p + 2)
                companion = prep_stream(p + 1, w_tiles[p + 1])
                if p == 0:
                    companion = chain(vt_stream(4, NKB), companion)
            else:
                companion = None
            drive(attn_stream(p), companion, ratio=2)
    return _legalize_waits(nc)


def _rope_tables():
    inv = 1.0 / (1000.0 ** (np.arange(0, HS, 2, dtype=np.float64) / HS))
    t = np.arange(T, dtype=np.float64)[:, None] * inv[None, :]
    sinT = np.sin(t).astype(np.float32).T  # [32, T]
    cosT = np.cos(t).astype(np.float32).T
    cosF = np.concatenate([cosT] * 4, 0)  # [128, T]
    # sign layout for multiply-BEFORE-swap: sq = q*sinF, swapped(sq) lands as
    # [-v*sin; +u*sin] in the [u; v] destination slots.
    sinF = np.concatenate([sinT, -sinT, sinT, -sinT], 0)
    return cosF, sinF


def _host_prep():
    cosF, sinF = _rope_tables()
    psw = np.zeros((P, P), np.float32)
    for hh in range(2):
        o = hh * HS
        psw[o : o + 32, o + 32 : o + 64] = np.eye(32)
        psw[o + 32 : o + 64, o : o + 32] = np.eye(32)
    # band[p, j] = 1 iff j >= p: causal triangle for the 128-wide diagonal
    # band, replicated for the two packed heads
    tri = np.tril(np.ones((P, P), np.float32)).T
    band = np.concatenate([tri, tri], axis=1)  # [P, 2P]
    return cosF, sinF, psw, band


def kernel(x, W_attn, b_attn, W_o, b_o, _trace=False, _tmpdir=None):
    x = np.asarray(x, np.float32)
    W_attn = np.asarray(W_attn, np.float32)
    b_attn = np.asarray(b_attn, np.float32)
    W_o = np.asarray(W_o, np.float32)
    b_o = np.asarray(b_o, np.float32)

    if "nc" not in _cache:
        _cache["nc"] = _build_nc()
    nc = _cache["nc"]

    cosF, sinF, psw, band = _host_prep()
    cosF_b, sinF_b = cosF.astype(NPBF), sinF.astype(NPBF)
    psw_b, band_b = psw.astype(NPBF), band.astype(NPBF)

    def head_rows(h):  # q-rows of head h, evens then odds
        base = h * HS
        return np.concatenate(
            [np.arange(base, base + HS, 2), np.arange(base + 1, base + HS, 2)]
        )

    in_maps = []
    for core in range(NCORES):
        b, hg = core // 2, core % 2
        heads = [hg * 8 + i for i in range(8)]
        qk_rows = []
        v_rows = []
        for p in range(NPAIR):
            h0, h1 = heads[2 * p], heads[2 * p + 1]
            qrows = np.concatenate([head_rows(h0), head_rows(h1)])
            qk_rows += [qrows, C + qrows]
            v_rows += [2 * C + np.concatenate(
                [np.arange(h0 * HS, (h0 + 1) * HS),
                 np.arange(h1 * HS, (h1 + 1) * HS)])]
        qk_rows = np.concatenate(qk_rows)  # [1024] pair-major (q,k) order
        v_rows = np.concatenate(v_rows)    # [512]  pair-major natural order
        # device-friendly swizzles: partition-major with contiguous per-
        # partition runs ([P, ...]) so DMAs move multi-KB packets
        wqkT = (
            W_attn[qk_rows].T.reshape(CB, P, NPAIR, 2 * P)
            .transpose(1, 2, 0, 3)
            .reshape(P, NPAIR * CB * 2 * P)
        )
        wqkT = np.ascontiguousarray(wqkT).astype(NPBF)
        bqk = np.ascontiguousarray(b_attn[qk_rows].reshape(8, P).T)  # [P, 8]
        wvT = (
            W_attn[v_rows].T.reshape(CB, P, 4 * P)
            .transpose(1, 0, 2)
            .reshape(P, CB * 4 * P)
        )
        wvT = np.ascontiguousarray(wvT).astype(NPBF)
        woT = (
            W_o[:, hg * 512 : (hg + 1) * 512].T.reshape(NPAIR, P, C)
            .transpose(1, 0, 2)
            .reshape(P, NPAIR * C)
        )
        woT = np.ascontiguousarray(woT).astype(NPBF)
        xT = (
            x[b].T.reshape(CB, P, NTC, QW)
            .transpose(1, 2, 0, 3)
            .reshape(P, NTC * CB * QW)
        )
        xT = np.ascontiguousarray(xT).astype(NPBF)
        in_maps.append(
            dict(xT=xT, wqkT=wqkT, bqk=bqk, wvT=wvT, woT=woT, cosF=cosF_b,
                 sinF=sinF_b, psw=psw_b, band=band_b)
        )

    res = run_bass_kernel_spmd(nc, in_maps, core_ids=list(range(NCORES)),
                               trace=_trace, tmpdir=_tmpdir)
    y = np.zeros((B, T, C), np.float32)
    for core in range(NCORES):
        y[core // 2] += res.results[core]["y"].astype(np.float32)
    # v-bias contribution: sum_k softmax_k (v_k + b_v) = (sum) + b_v, so
    # y gains the constant row b_v @ W_o^T; fold it in with b_o here.
    b_v = b_attn[2 * C :]
    y += (W_o @ b_v + b_o)[None, None, :]
    if _trace:
        _cache["last_result"] = res
    return y


# revision 28
# speedup vs baseline: 1.2987x; 1.0461x over previous
"""Causal self-attention (B=4, T=2048, C=1024, NH=16, HS=64) on 8 trn2 cores.

Sharding: core = (batch b, head-group hg): b = core//2, hg = core%2.
Each core computes 8 heads of one batch: column-parallel W_attn (its heads'
q/k rows), row-parallel W_o (its heads' columns).  Host sums the two
head-group partials per batch and adds b_o (+ W_o @ b_v: softmax weights sum
to 1, so the v-bias contributes a constant row -> folded into the host bias).

Device algorithm (per core, all matmuls bf16 inputs / fp32 PSUM):
  q^T,k^T = W_local @ x^T          (transposed layout [j, t])
  rope via q*cosF + (P_swap @ (q*sinF_signed))   (P_swap = const permutation)
  v      = (x @ W_v^T) directly in [t, d] layout (lhsT = x^T block), with
           ones columns interleaved per head -> no PE transposes, no v bias
  S^T[k,q] = K_rot^T.T @ Q_rot^T   (scores transposed, causal blocks only,
           diagonal blocks restricted to columns >= off: no wasted PE cols)
  E = exp(S^T/8) (ScalarE, fused 1/8 scale), staircase mask on diagonal
  O^T|sums = [V|1]^T.T @ E         (fused unnormalized output + denominator,
           also column-restricted on diagonal blocks)
  O_norm = O^T * (1/sums)          (DVE recip, TensorE rank-1 broadcast)
  y = O_cat^T.T @ W_o_cols^T       (accumulate over head pairs in PSUM),
           written back as bf16 (host accumulates partials in fp32)

Perf structure (v3):
  - Two heads of a pair on partition halves 0-63/64-127: QK^T score matmuls
    run on concurrent PE row tiles; one fused [128, 2*512] exp per job.
  - Diagonal-block scores/AV matmuls only cover columns [off:]; the
    fully-masked region is never computed (saves ~37k PE cycles/core) and
    the es zero-memsets disappear.
  - V^T is computed straight from the projection (lhsT = x^T 128-block), so
    the 128 per-pair PE transposes + v bias adds of v2 are gone.
  - First weight/x DMA pieces are cb-pair sized so the first matmul starts
    ~5us in; bias rides first on the gpsimd queue; cos/sin land before the
    first rope chunk; wo on the scalar queue.
  - y stored bf16 (halves writeback bytes; host sums partials in fp32).

Head-dim channels are reordered on the host (per head: even dims then odd
dims) so RoPE pairs live in contiguous 32-partition blocks; attention scores
are invariant to this permutation since q and k use the same order, and v/W_o
stay in natural order.
"""

from contextlib import ExitStack
from itertools import chain

import numpy as np
import ml_dtypes

import concourse.bass as bass
import concourse.mybir as mybir
import concourse.tile as tile
from concourse.bass_utils import run_bass_kernel_spmd
from concourse.masks import make_identity

B, T, C = 4, 2048, 1024
NH, HS = 16, 64
P = 128
NCORES = 8
NPAIR = 4            # head pairs per core (8 local heads)
CB = C // P          # 8 contraction blocks over C
QW = 512             # q-chunk width
NTC = T // QW        # 4 q-chunks
NKB = T // P         # 16 key blocks
F32 = mybir.dt.float32
BF16 = mybir.dt.bfloat16
NPBF = ml_dtypes.bfloat16
AF = mybir.ActivationFunctionType
ALU = mybir.AluOpType

_cache = {}


def _legalize_waits(nc, max_waits=1):
    """The walrus build here allows only one sync-wait command per
    instruction; move excess Tile-generated waits onto preceding
    single-wait NoOps on the same engine (same-engine program order
    makes this equivalent)."""
    n_id = [0]
    for fn in nc.m.functions:
        for blk in fn.blocks:
            out = []
            for inst in blk.instructions:
                si = inst.sync_info
                if si is not None and si.on_wait and len(si.on_wait) > max_waits:
                    waits = list(si.on_wait)
                    excess, keep = waits[:-max_waits], waits[-max_waits:]
                    for w in excess:
                        n_id[0] += 1
                        out.append(
                            mybir.InstNoOp(
                                name=f"waitsplit-{n_id[0]}",
                                engine=inst.engine,
                                bass_nofuse=True,
                                sync_info=mybir.SyncInfo(
                                    on_wait=[w], on_update=[]
                                ),
                            )
                        )
                    inst.sync_info = mybir.SyncInfo(
                        on_wait=keep, on_update=list(si.on_update)
                    )
                out.append(inst)
            blk.instructions = out
    return nc


def _build_nc():
    nc = bass.Bass(target_bir_lowering=True)
    # all large inputs are pre-swizzled on the host so every DMA reads
    # contiguous multi-KB runs per partition (small-packet strided DMAs
    # run at a fraction of queue bandwidth)
    xT_d = nc.dram_tensor("xT", [P, NTC * CB * QW], BF16, kind="ExternalInput")
    w_d = nc.dram_tensor("wqkT", [P, NPAIR * CB * 2 * P], BF16,
                         kind="ExternalInput")
    b_d = nc.dram_tensor("bqk", [P, 8], F32, kind="ExternalInput")
    wv_d = nc.dram_tensor("wvT", [P, CB * 4 * P], BF16, kind="ExternalInput")
    wo_d = nc.dram_tensor("woT", [P, NPAIR * C], BF16, kind="ExternalInput")
    cos_d = nc.dram_tensor("cosF", [P, T], BF16, kind="ExternalInput")
    sin_d = nc.dram_tensor("sinF", [P, T], BF16, kind="ExternalInput")
    psw_d = nc.dram_tensor("psw", [P, P], BF16, kind="ExternalInput")
    band_d = nc.dram_tensor("band", [P, 2 * P], BF16, kind="ExternalInput")
    y_d = nc.dram_tensor("y", [T, C], BF16, kind="ExternalOutput")

    with tile.TileContext(nc) as tc, ExitStack() as ctx:
        const = ctx.enter_context(tc.tile_pool(name="const", bufs=1))
        wpool = ctx.enter_context(tc.tile_pool(name="wpool", bufs=2))
        qkpool = ctx.enter_context(tc.tile_pool(name="qkpool", bufs=4))
        tmppool = ctx.enter_context(tc.tile_pool(name="tmppool", bufs=2))
        rotpool = ctx.enter_context(tc.tile_pool(name="rotpool", bufs=4))
        epool = ctx.enter_context(tc.tile_pool(name="epool", bufs=8))
        extpool = ctx.enter_context(tc.tile_pool(name="extpool", bufs=3))
        rcppool = ctx.enter_context(tc.tile_pool(name="rcppool", bufs=2))
        opool = ctx.enter_context(tc.tile_pool(name="opool", bufs=4))
        ypool = ctx.enter_context(tc.tile_pool(name="ypool", bufs=2))
        # PSUM budget (8 banks): qk 2x[P,2,QW]=4, av 2x[65,QW]=2, misc 2x[P,QW]=2
        ps_qk = ctx.enter_context(tc.tile_pool(name="ps_qk", bufs=2, space="PSUM"))
        ps_av = ctx.enter_context(tc.tile_pool(name="ps_av", bufs=2, space="PSUM"))
        ps_m = ctx.enter_context(tc.tile_pool(name="ps_m", bufs=2, space="PSUM"))

        # ---- loads, ordered so pair-0 compute can start within ~5us ----
        def load_w(w_sb, p, splits=(8,)):
            base = p * CB * 2 * P
            cb0 = 0
            for npc in splits:
                sl = slice(base + cb0 * 2 * P, base + (cb0 + npc) * 2 * P)
                nc.sync.dma_start(
                    w_sb[:, cb0 : cb0 + npc, :],
                    w_d[:, sl].rearrange("p (cb j) -> p cb j", cb=npc),
                )
                cb0 += npc

        w0_sb = wpool.tile([P, CB, 2 * P], BF16, tag="w", name="w_sb")
        load_w(w0_sb, 0, splits=(1, 1, 2, 4))

        xT_sb = []
        for tc_i in range(NTC):
            xT_sb.append(const.tile([P, CB, QW], BF16, name=f"xT{tc_i}",
                                    tag=f"xT{tc_i}"))

        def load_x(tc_i, eng, splits=(4, 4)):
            xt = xT_sb[tc_i]
            base = tc_i * CB * QW
            cb0 = 0
            for npc in splits:
                sl = slice(base + cb0 * QW, base + (cb0 + npc) * QW)
                eng.dma_start(
                    xt[:, cb0 : cb0 + npc, :],
                    xT_d[:, sl].rearrange("p (cb q) -> p cb q", cb=npc),
                )
                cb0 += npc

        # scalar queue: xT0 (fine pieces), xT2, wo (needed late)
        load_x(0, nc.scalar, splits=(1, 1, 2, 4))
        # gpsimd queue: bias (tiny, needed by first proj output), xT1,
        # psw+cos+sin (first rope chunk ~18us), xT3, band (first diag exp)
        bias_sb = const.tile([P, 8], F32)
        nc.gpsimd.dma_start(bias_sb[:], b_d[:])
        load_x(1, nc.gpsimd, splits=(2, 2, 4))
        load_x(2, nc.scalar, splits=(4, 4))
        psw_sb = const.tile([P, P], BF16)
        nc.gpsimd.dma_start(psw_sb[:], psw_d[:])
        cos_sb = const.tile([P, T], BF16)
        sin_sb = const.tile([P, T], BF16)
        nc.gpsimd.dma_start(cos_sb[:, : T // 2], cos_d[:, : T // 2])
        nc.gpsimd.dma_start(sin_sb[:, : T // 2], sin_d[:, : T // 2])
        # sync queue (after pair-0 qk weights): xT3, then v weights for the
        # direct v^T projection (first vt block needs all cb of wv)
        load_x(3, nc.sync, splits=(4, 4))
        wv_sb = const.tile([P, CB, 4 * P], BF16)
        for i in range(2):
            sl = slice(i * 4 * 4 * P, (i + 1) * 4 * 4 * P)
            nc.sync.dma_start(
                wv_sb[:, i * 4 : (i + 1) * 4, :],
                wv_d[:, sl].rearrange("p (cb j) -> p cb j", cb=4),
            )
        nc.gpsimd.dma_start(cos_sb[:, T // 2 :], cos_d[:, T // 2 :])
        nc.gpsimd.dma_start(sin_sb[:, T // 2 :], sin_d[:, T // 2 :])
        band_sb = const.tile([P, 2, P], BF16)
        nc.gpsimd.dma_start(band_sb[:],
                            band_d.rearrange("p (g q) -> p g q", g=2))
        wo_sb = const.tile([P, NPAIR, C], BF16)
        nc.scalar.dma_start(wo_sb[:], wo_d.rearrange("p (pr o) -> p pr o",
                                                     pr=NPAIR))
        ident = const.tile([P, P], BF16)
        make_identity(nc, ident[:])
        ones64 = const.tile([65, HS], BF16)
        nc.gpsimd.memset(ones64[64:65, :], 1.0)

        # v in [t, d] layout: vn[:, kb, p, h, 0:64] = v dims, [..., 64] = 1.0
        # (interleaved ones columns keep each head's [P, 65] lhsT contiguous)
        vn = const.tile([P, NKB, NPAIR, 2, HS + 1], BF16, name="vn")
        nc.gpsimd.memset(vn[:, :, :, :, HS : HS + 1], 1.0)

        ocat = [opool.tile([P, T], BF16, name=f"ocat{p}", tag="ocat")
                for p in range(NPAIR)]

        prep_out = {}

        def prep_stream(p, w_sb):
            """q/k projection + rope for pair p, chunk-major (each x chunk
            is fully consumed before the next is touched, which keeps early
            PE demand under the DMA feed rate).  Yields between PE-sized
            chunks; emission order sets scheduler priority so this work
            fills pair p-1's attention exp-wait bubbles."""
            qk = [qkpool.tile([P, T], BF16, tag="qkT", name="qkT")
                  for _ in range(2)]
            rots = [rotpool.tile([P, T], BF16, tag="rot", name="rot")
                    for _ in range(2)]
            for tc_i in range(NTC):
                sl = slice(tc_i * QW, (tc_i + 1) * QW)
                for jb in range(2):
                    dst = qk[jb]
                    bias_bc = bias_sb[
                        :, 2 * p + jb : 2 * p + jb + 1
                    ].to_broadcast((P, QW))
                    psum = ps_m.tile([P, QW], F32, tag="m", name="pj")
                    for cb in range(CB):
                        nc.tensor.matmul(
                            psum[:],
                            lhsT=w_sb[:, cb, jb * P : (jb + 1) * P],
                            rhs=xT_sb[tc_i][:, cb, :],
                            start=(cb == 0),
                            stop=(cb == CB - 1),
                        )
                    nc.vector.tensor_tensor(
                        dst[:, sl], psum[:], bias_bc, ALU.add,
                    )
                    yield
                for jb in range(2):  # rot = t*cos + P_swap @ (t*sin_signed)
                    src, rot = qk[jb], rots[jb]
                    sq = tmppool.tile([P, QW], BF16, tag="sq", name="sq")
                    nc.vector.tensor_tensor(
                        sq[:], src[:, sl], sin_sb[:, sl], ALU.mult
                    )
                    nc.vector.tensor_tensor(
                        rot[:, sl], src[:, sl], cos_sb[:, sl], ALU.mult
                    )
                    psum = ps_m.tile([P, QW], F32, tag="m", name="sw")
                    nc.tensor.matmul(
                        psum[:],
                        lhsT=psw_sb[:],
                        rhs=sq[:],
                        start=True,
                        stop=True,
                    )
                    nc.vector.tensor_tensor(
                        rot[:, sl], rot[:, sl], psum[:], ALU.add,
                    )
                    yield
            prep_out[p] = (rots[0], rots[1])

        def vt_stream(tb_lo, tb_hi):
            """v^T for 128-row blocks tb_lo..tb_hi-1, all pairs at once:
            psum[t, (p,h,d)] = sum_c x^T[c, t-block]^T wv^T[c, (p,h,d)]."""
            for tb in range(tb_lo, tb_hi):
                tci, loc = tb // 4, tb % 4
                psum = ps_m.tile([P, QW], F32, tag="m", name="vt")
                for cb in range(CB):
                    nc.tensor.matmul(
                        psum[:],
                        lhsT=xT_sb[tci][:, cb, loc * P : (loc + 1) * P],
                        rhs=wv_sb[:, cb, :],
                        start=(cb == 0),
                        stop=(cb == CB - 1),
                    )
                nc.vector.tensor_copy(
                    vn[:, tb, :, :, :HS],
                    psum[:].rearrange("p (pr h d) -> p pr h d", pr=NPAIR, h=2),
                )
                yield

        def emit_outproj_block(tb):
            """y[tb*P:(tb+1)*P, :] = sum_p ocat_p^T @ woT_p for one 128-row
            block; emitted as soon as all pairs' ocat columns are final.
            One row-contiguous DMA per block keeps the writeback (and the
            end-of-kernel DMA-semaphore sweep) at 16 transfers."""
            yb = ypool.tile([P, 2, QW], BF16, tag="yb")
            for oc in range(2):
                psum = ps_m.tile([P, QW], F32, tag="m", name="yp")
                for p in range(NPAIR):
                    nc.tensor.matmul(
                        psum[:],
                        lhsT=ocat[p][:, tb * P : (tb + 1) * P],
                        rhs=wo_sb[:, p, oc * QW : (oc + 1) * QW],
                        start=(p == 0),
                        stop=(p == NPAIR - 1),
                    )
                nc.vector.tensor_copy(yb[:, oc, :], psum[:])
            eng = nc.sync if tb % 2 == 0 else nc.gpsimd
            eng.dma_start(
                y_d[tb * P : (tb + 1) * P, :],
                yb[:].rearrange("p g q -> p (g q)"),
            )

        def attn_stream(p):
            """Attention for pair p.  Jobs are packed per (qc, kb): both
            heads' QK^T matmuls are emitted back-to-back (concurrent PE row
            tiles 0-63 / 64-127), followed by one fused exp over both heads'
            PSUM banks and the two AV accumulations.  Diagonal blocks only
            compute columns [off:]."""
            rq, rk = prep_out[p]
            ps_o_cur = {}

            def normalize(ext2, qc, heads):
                # 1/sums: ln+exp on the partition-64 sums row (same ACT
                # table as the scores exp -> no table reload)
                rcpf = rcppool.tile([65, 2, QW], F32, tag="rcpf", name="rcpf")
                nc.scalar.activation(
                    rcpf[64:65, heads, :], ext2[64:65, heads, :], AF.Ln
                )
                rcpb = rcppool.tile([65, 2, QW], BF16, tag="rcpb", name="rcpb")
                nc.scalar.activation(
                    rcpb[64:65, heads, :], rcpf[64:65, heads, :],
                    AF.Exp, scale=-1.0,
                )
                hs = range(2) if heads == slice(None) else [heads]
                for h in hs:
                    rb = ps_m.tile([HS, QW], F32, tag="m", name="rb")
                    nc.tensor.matmul(
                        rb[:],
                        lhsT=ones64[64:65, :],
                        rhs=rcpb[64:65, h, :],
                        start=True,
                        stop=True,
                    )
                    nc.vector.tensor_tensor(
                        ocat[p][h * HS : (h + 1) * HS,
                                qc * QW : (qc + 1) * QW],
                        ext2[:HS, h, :],
                        rb[:],
                        ALU.mult,
                    )

            for qc in range(NTC):
                nkb = 4 * (qc + 1)
                for h in range(2):
                    ps_o_cur[h] = ps_av.tile([HS + 1, QW], F32, tag="av",
                                             name="ps_o")
                for kb in range(nkb):
                    off = max(0, P * (kb - 4 * qc))
                    ps_s = ps_qk.tile([P, 2, QW], F32, tag="qk", name="ps_s")
                    for h in range(2):
                        nc.tensor.matmul(
                            ps_s[:, h, off:],
                            lhsT=rk[h * HS : (h + 1) * HS,
                                    kb * P : (kb + 1) * P],
                            rhs=rq[h * HS : (h + 1) * HS,
                                   qc * QW + off : (qc + 1) * QW],
                            start=True,
                            stop=True,
                        )
                    es = epool.tile([P, 2, QW], BF16, tag="es", name="es")
                    if off == 0:
                        nc.scalar.activation(
                            es[:].rearrange("p g q -> p (g q)"),
                            ps_s[:].rearrange("p g q -> p (g q)"),
                            AF.Exp,
                            scale=0.125,
                        )
                    else:  # diagonal block: partial exp + staircase mask
                        nc.scalar.activation(
                            es[:, :, off:], ps_s[:, :, off:],
                            AF.Exp, scale=0.125,
                        )
                    if kb - 4 * qc >= 0:
                        nc.vector.tensor_tensor(
                            es[:, :, off : off + P],
                            es[:, :, off : off + P],
                            band_sb[:],
                            ALU.mult,
                        )
                    # Late priority: when several PE instructions are ready
                    # the scheduler prefers QKT/prep work, so AVs trail exp
                    # by as much as the es pool depth allows (robust to
                    # cost-model vs hardware timing skew).
                    with tc.high_priority(offset=-1_000_000):
                        for h in range(2):
                            nc.tensor.matmul(
                                ps_o_cur[h][:, off:],
                                lhsT=vn[:, kb, p, h, :],
                                rhs=es[:, h, off:],
                                start=(kb == 0),
                                stop=(kb == nkb - 1),
                            )
                    yield
                # pair 3's normalize feeds the out-projection: keep it eager
                # and per-head there (shortest latency to the first rank-1);
                # defer it elsewhere (nothing reads ocat until pair 3)
                if p < NPAIR - 1:
                    # the copies free the ps_av banks for the next qc: keep
                    # them at normal priority; only the ln/exp/rank-1/mult
                    # (read ocat no earlier than pair 3) are deferred
                    ext2 = extpool.tile([HS + 1, 2, QW], F32, tag="ext",
                                        name="ext2")
                    for h in range(2):
                        nc.vector.tensor_copy(ext2[:, h, :],
                                              ps_o_cur[h][:])
                    with tc.high_priority(offset=-1_000_000):
                        normalize(ext2, qc, slice(None))
                else:
                    ext2 = extpool.tile([HS + 1, 2, QW], F32, tag="ext",
                                        name="ext2")
                    for h in range(2):
                        nc.vector.tensor_copy(ext2[:, h, :], ps_o_cur[h][:])
                        normalize(ext2, qc, h)
                yield
                if p == NPAIR - 1:
                    with tc.high_priority(offset=-1_000_000):
                        for tb in range(4 * qc, 4 * qc + 4):
                            emit_outproj_block(tb)
                    yield

        def drive(a_gen, b_gen, ratio=2):
            done_a = a_gen is None
            done_b = b_gen is None
            while not (done_a and done_b):
                if not done_a:
                    for _ in range(ratio):
                        try:
                            next(a_gen)
                        except StopIteration:
                            done_a = True
                            break
                if not done_b:
                    try:
                        next(b_gen)
                    except StopIteration:
                        done_b = True

        w_tiles = {0: w0_sb}
        w_tiles[1] = wpool.tile([P, CB, 2 * P], BF16, tag="w", name="w_sb")
        load_w(w_tiles[1], 1)
        for _ in prep_stream(0, w0_sb):
            pass
        for _ in vt_stream(0, 4):
            pass
        for p in range(NPAIR):
            if p + 1 < NPAIR:
                if p + 2 < NPAIR:
                    w_tiles[p + 2] = wpool.tile([P, CB, 2 * P], BF16,
                                                tag="w", name="w_sb")
                    load_w(w_tiles[p + 2], p + 2)
                companion = prep_stream(p + 1, w_tiles[p + 1])
                if p == 0:
                    companion = chain(vt_stream(4, NKB), companion)
            else:
                companion = None
            drive(attn_stream(p), companion, ratio=1 if p == 0 else 2)
    return _legalize_waits(nc)


def _rope_tables():
    inv = 1.0 / (1000.0 ** (np.arange(0, HS, 2, dtype=np.float64) / HS))
    t = np.arange(T, dtype=np.float64)[:, None] * inv[None, :]
    sinT = np.sin(t).astype(np.float32).T  # [32, T]
    cosT = np.cos(t).astype(np.float32).T
    cosF = np.concatenate([cosT] * 4, 0)  # [128, T]
    # sign layout for multiply-BEFORE-swap: sq = q*sinF, swapped(sq) lands as
    # [-v*sin; +u*sin] in the [u; v] destination slots.
    sinF = np.concatenate([sinT, -sinT, sinT, -sinT], 0)
    return cosF, sinF


def _host_prep():
    cosF, sinF = _rope_tables()
    psw = np.zeros((P, P), np.float32)
    for hh in range(2):
        o = hh * HS
        psw[o : o + 32, o + 32 : o + 64] = np.eye(32)
        psw[o + 32 : o + 64, o : o + 32] = np.eye(32)
    # band[p, j] = 1 iff j >= p: causal triangle for the 128-wide diagonal
    # band, replicated for the two packed heads
    tri = np.tril(np.ones((P, P), np.float32)).T
    band = np.concatenate([tri, tri], axis=1)  # [P, 2P]
    return cosF, sinF, psw, band


def kernel(x, W_attn, b_attn, W_o, b_o, _trace=False, _tmpdir=None):
    x = np.asarray(x, np.float32)
    W_attn = np.asarray(W_attn, np.float32)
    b_attn = np.asarray(b_attn, np.float32)
    W_o = np.asarray(W_o, np.float32)
    b_o = np.asarray(b_o, np.float32)

    if "nc" not in _cache:
        _cache["nc"] = _build_nc()
    nc = _cache["nc"]

    cosF, sinF, psw, band = _host_prep()
    cosF_b, sinF_b = cosF.astype(NPBF), sinF.astype(NPBF)
    psw_b, band_b = psw.astype(NPBF), band.astype(NPBF)

    def head_rows(h):  # q-rows of head h, evens then odds
        base = h * HS
        return np.concatenate(
            [np.arange(base, base + HS, 2), np.arange(base + 1, base + HS, 2)]
        )

    in_maps = []
    for core in range(NCORES):
        b, hg = core // 2, core % 2
        heads = [hg * 8 + i for i in range(8)]
        qk_rows = []
        v_rows = []
        for p in range(NPAIR):
            h0, h1 = heads[2 * p], heads[2 * p + 1]
            qrows = np.concatenate([head_rows(h0), head_rows(h1)])
            qk_rows += [qrows, C + qrows]
            v_rows += [2 * C + np.concatenate(
                [np.arange(h0 * HS, (h0 + 1) * HS),
                 np.arange(h1 * HS, (h1 + 1) * HS)])]
        qk_rows = np.concatenate(qk_rows)  # [1024] pair-major (q,k) order
        v_rows = np.concatenate(v_rows)    # [512]  pair-major natural order
        # device-friendly swizzles: partition-major with contiguous per-
        # partition runs ([P, ...]) so DMAs move multi-KB packets
        wqkT = (
            W_attn[qk_rows].T.reshape(CB, P, NPAIR, 2 * P)
            .transpose(1, 2, 0, 3)
            .reshape(P, NPAIR * CB * 2 * P)
        )
        wqkT = np.ascontiguousarray(wqkT).astype(NPBF)
        bqk = np.ascontiguousarray(b_attn[qk_rows].reshape(8, P).T)  # [P, 8]
        wvT = (
            W_attn[v_rows].T.reshape(CB, P, 4 * P)
            .transpose(1, 0, 2)
            .reshape(P, CB * 4 * P)
        )
        wvT = np.ascontiguousarray(wvT).astype(NPBF)
        woT = (
            W_o[:, hg * 512 : (hg + 1) * 512].T.reshape(NPAIR, P, C)
            .transpose(1, 0, 2)
            .reshape(P, NPAIR * C)
        )
        woT = np.ascontiguousarray(woT).astype(NPBF)
        xT = (
            x[b].T.reshape(CB, P, NTC, QW)
            .transpose(1, 2, 0, 3)
            .reshape(P, NTC * CB * QW)
        )
        xT = np.ascontiguousarray(xT).astype(NPBF)
        in_maps.append(
            dict(xT=xT, wqkT=wqkT, bqk=bqk, wvT=wvT, woT=woT, cosF=cosF_b,
                 sinF=sinF_b, psw=psw_b, band=band_b)
        )

    res = run_bass_kernel_spmd(nc, in_maps, core_ids=list(range(NCORES)),
                               trace=_trace, tmpdir=_tmpdir)
    y = np.zeros((B, T, C), np.float32)
    for core in range(NCORES):
        y[core // 2] += res.results[core]["y"].astype(np.float32)
    # v-bias contribution: sum_k softmax_k (v_k + b_v) = (sum) + b_v, so
    # y gains the constant row b_v @ W_o^T; fold it in with b_o here.
    b_v = b_attn[2 * C :]
    y += (W_o @ b_v + b_o)[None, None, :]
    if _trace:
        _cache["last_result"] = res
    return y


# revision 33
# speedup vs baseline: 1.3201x; 1.0165x over previous
"""Causal self-attention (B=4, T=2048, C=1024, NH=16, HS=64) on 8 trn2 cores.

Sharding: core = (batch b, head-group hg): b = core//2, hg = core%2.
Each core computes 8 heads of one batch: column-parallel W_attn (its heads'
q/k rows), row-parallel W_o (its heads' columns).  Host sums the two
head-group partials per batch and adds b_o (+ W_o @ b_v: softmax weights sum
to 1, so the v-bias contributes a constant row -> folded into the host bias).

Device algorithm (per core, all matmuls bf16 inputs / fp32 PSUM):
  q^T,k^T = W_local @ x^T          (transposed layout [j, t])
  rope via q*cosF + (P_swap @ (q*sinF_signed))   (P_swap = const permutation)
  v      = (x @ W_v^T) directly in [t, d] layout (lhsT = x^T block), with
           ones columns interleaved per head -> no PE transposes, no v bias
  S^T[k,q] = K_rot^T.T @ Q_rot^T   (scores transposed, causal blocks only,
           diagonal blocks restricted to columns >= off: no wasted PE cols)
  E = exp(S^T/8) (ScalarE, fused 1/8 scale), staircase mask on diagonal
  O^T|sums = [V|1]^T.T @ E         (fused unnormalized output + denominator,
           also column-restricted on diagonal blocks)
  O_norm = O^T * (1/sums)          (DVE recip, TensorE rank-1 broadcast)
  y = O_cat^T.T @ W_o_cols^T       (accumulate over head pairs in PSUM),
           written back as bf16 (host accumulates partials in fp32)

Perf structure (v3):
  - Two heads of a pair on partition halves 0-63/64-127: QK^T score matmuls
    run on concurrent PE row tiles; one fused [128, 2*512] exp per job.
  - Diagonal-block scores/AV matmuls only cover columns [off:]; the
    fully-masked region is never computed (saves ~37k PE cycles/core) and
    the es zero-memsets disappear.
  - V^T is computed straight from the projection (lhsT = x^T 128-block), so
    the 128 per-pair PE transposes + v bias adds of v2 are gone.
  - First weight/x DMA pieces are cb-pair sized so the first matmul starts
    ~5us in; bias rides first on the gpsimd queue; cos/sin land before the
    first rope chunk; wo on the scalar queue.
  - y stored bf16 (halves writeback bytes; host sums partials in fp32).

Head-dim channels are reordered on the host (per head: even dims then odd
dims) so RoPE pairs live in contiguous 32-partition blocks; attention scores
are invariant to this permutation since q and k use the same order, and v/W_o
stay in natural order.
"""

from contextlib import ExitStack
from itertools import chain

import numpy as np
import ml_dtypes

import concourse.bass as bass
import concourse.mybir as mybir
import concourse.tile as tile
from concourse.bass_utils import run_bass_kernel_spmd
from concourse.masks import make_identity

B, T, C = 4, 2048, 1024
NH, HS = 16, 64
P = 128
NCORES = 8
NPAIR = 4            # head pairs per core (8 local heads)
CB = C // P          # 8 contraction blocks over C
QW = 512             # q-chunk width
NTC = T // QW        # 4 q-chunks
NKB = T // P         # 16 key blocks
F32 = mybir.dt.float32
BF16 = mybir.dt.bfloat16
NPBF = ml_dtypes.bfloat16
AF = mybir.ActivationFunctionType
ALU = mybir.AluOpType

_cache = {}


def _legalize_waits(nc, max_waits=1):
    """The walrus build here allows only one sync-wait command per
    instruction; move excess Tile-generated waits onto preceding
    single-wait NoOps on the same engine (same-engine program order
    makes this equivalent)."""
    n_id = [0]
    for fn in nc.m.functions:
        for blk in fn.blocks:
            out = []
            for inst in blk.instructions:
                si = inst.sync_info
                if si is not None and si.on_wait and len(si.on_wait) > max_waits:
                    waits = list(si.on_wait)
                    excess, keep = waits[:-max_waits], waits[-max_waits:]
                    for w in excess:
                        n_id[0] += 1
                        out.append(
                            mybir.InstNoOp(
                                name=f"waitsplit-{n_id[0]}",
                                engine=inst.engine,
                                bass_nofuse=True,
                                sync_info=mybir.SyncInfo(
                                    on_wait=[w], on_update=[]
                                ),
                            )
                        )
                    inst.sync_info = mybir.SyncInfo(
                        on_wait=keep, on_update=list(si.on_update)
                    )
                out.append(inst)
            blk.instructions = out
    return nc


def _build_nc():
    nc = bass.Bass(target_bir_lowering=True)
    # all large inputs are pre-swizzled on the host so every DMA reads
    # contiguous multi-KB runs per partition (small-packet strided DMAs
    # run at a fraction of queue bandwidth)
    xT_d = nc.dram_tensor("xT", [P, NTC * CB * QW], BF16, kind="ExternalInput")
    w_d = nc.dram_tensor("wqkT", [P, NPAIR * CB * 2 * P], BF16,
                         kind="ExternalInput")
    b_d = nc.dram_tensor("bqk", [P, 8], F32, kind="ExternalInput")
    wv_d = nc.dram_tensor("wvT", [P, CB * 4 * P], BF16, kind="ExternalInput")
    wo_d = nc.dram_tensor("woT", [P, NPAIR * C], BF16, kind="ExternalInput")
    cos_d = nc.dram_tensor("cosF", [P, T], BF16, kind="ExternalInput")
    sin_d = nc.dram_tensor("sinF", [P, T], BF16, kind="ExternalInput")
    psw_d = nc.dram_tensor("psw", [P, P], BF16, kind="ExternalInput")
    band_d = nc.dram_tensor("band", [P, 2 * P], BF16, kind="ExternalInput")
    y_d = nc.dram_tensor("y", [T, C], BF16, kind="ExternalOutput")

    with tile.TileContext(nc) as tc, ExitStack() as ctx:
        const = ctx.enter_context(tc.tile_pool(name="const", bufs=1))
        wpool = ctx.enter_context(tc.tile_pool(name="wpool", bufs=2))
        qkpool = ctx.enter_context(tc.tile_pool(name="qkpool", bufs=4))
        tmppool = ctx.enter_context(tc.tile_pool(name="tmppool", bufs=2))
        rotpool = ctx.enter_context(tc.tile_pool(name="rotpool", bufs=4))
        epool = ctx.enter_context(tc.tile_pool(name="epool", bufs=8))
        extpool = ctx.enter_context(tc.tile_pool(name="extpool", bufs=3))
        rcppool = ctx.enter_context(tc.tile_pool(name="rcppool", bufs=2))
        opool = ctx.enter_context(tc.tile_pool(name="opool", bufs=4))
        ypool = ctx.enter_context(tc.tile_pool(name="ypool", bufs=2))
        # PSUM budget (8 banks): qk 2x[P,2,QW]=4, av 2x[65,QW]=2, misc 2x[P,QW]=2
        ps_qk = ctx.enter_context(tc.tile_pool(name="ps_qk", bufs=2, space="PSUM"))
        ps_av = ctx.enter_context(tc.tile_pool(name="ps_av", bufs=2, space="PSUM"))
        ps_m = ctx.enter_context(tc.tile_pool(name="ps_m", bufs=2, space="PSUM"))

        # ---- loads, ordered so pair-0 compute can start within ~5us ----
        def load_w(w_sb, p, splits=(8,), engine=None):
            eng = engine or nc.sync
            base = p * CB * 2 * P
            cb0 = 0
            for npc in splits:
                sl = slice(base + cb0 * 2 * P, base + (cb0 + npc) * 2 * P)
                eng.dma_start(
                    w_sb[:, cb0 : cb0 + npc, :],
                    w_d[:, sl].rearrange("p (cb j) -> p cb j", cb=npc),
                )
                cb0 += npc

        w0_sb = wpool.tile([P, CB, 2 * P], BF16, tag="w", name="w_sb")
        load_w(w0_sb, 0, splits=(1, 1, 2, 4))

        xT_sb = []
        for tc_i in range(NTC):
            xT_sb.append(const.tile([P, CB, QW], BF16, name=f"xT{tc_i}",
                                    tag=f"xT{tc_i}"))

        def load_x(tc_i, eng, splits=(4, 4)):
            xt = xT_sb[tc_i]
            base = tc_i * CB * QW
            cb0 = 0
            for npc in splits:
                sl = slice(base + cb0 * QW, base + (cb0 + npc) * QW)
                eng.dma_start(
                    xt[:, cb0 : cb0 + npc, :],
                    xT_d[:, sl].rearrange("p (cb q) -> p cb q", cb=npc),
                )
                cb0 += npc

        # scalar queue: xT0 (fine pieces), xT2, wo (needed late)
        load_x(0, nc.scalar, splits=(1, 1, 2, 4))
        # gpsimd queue: bias (tiny, needed by first proj output), xT1,
        # psw+cos+sin (first rope chunk ~18us), xT3, band (first diag exp)
        bias_sb = const.tile([P, 8], F32)
        nc.gpsimd.dma_start(bias_sb[:], b_d[:])
        load_x(1, nc.gpsimd, splits=(2, 2, 4))
        load_x(2, nc.scalar, splits=(4, 4))
        psw_sb = const.tile([P, P], BF16)
        nc.gpsimd.dma_start(psw_sb[:], psw_d[:])
        cos_sb = const.tile([P, T], BF16)
        sin_sb = const.tile([P, T], BF16)
        nc.gpsimd.dma_start(cos_sb[:, : T // 2], cos_d[:, : T // 2])
        nc.gpsimd.dma_start(sin_sb[:, : T // 2], sin_d[:, : T // 2])
        # sync queue (after pair-0 qk weights): xT3, then v weights for the
        # direct v^T projection (first vt block needs all cb of wv)
        load_x(3, nc.sync, splits=(4, 4))
        wv_sb = const.tile([P, CB, 4 * P], BF16)
        for i in range(2):
            sl = slice(i * 4 * 4 * P, (i + 1) * 4 * 4 * P)
            nc.sync.dma_start(
                wv_sb[:, i * 4 : (i + 1) * 4, :],
                wv_d[:, sl].rearrange("p (cb j) -> p cb j", cb=4),
            )
        nc.gpsimd.dma_start(cos_sb[:, T // 2 :], cos_d[:, T // 2 :])
        nc.gpsimd.dma_start(sin_sb[:, T // 2 :], sin_d[:, T // 2 :])
        band_sb = const.tile([P, 2, P], BF16)
        nc.gpsimd.dma_start(band_sb[:],
                            band_d.rearrange("p (g q) -> p g q", g=2))
        wo_sb = const.tile([P, NPAIR, C], BF16)
        nc.scalar.dma_start(wo_sb[:], wo_d.rearrange("p (pr o) -> p pr o",
                                                     pr=NPAIR))
        ident = const.tile([P, P], BF16)
        make_identity(nc, ident[:])
        ones64 = const.tile([65, HS], BF16)
        nc.gpsimd.memset(ones64[64:65, :], 1.0)

        # v in [t, d] layout: vn[:, kb, p, h, 0:64] = v dims, [..., 64] = 1.0
        # (interleaved ones columns keep each head's [P, 65] lhsT contiguous)
        vn = const.tile([P, NKB, NPAIR, 2, HS + 1], BF16, name="vn")
        nc.gpsimd.memset(vn[:, :, :, :, HS : HS + 1], 1.0)

        ocat = [opool.tile([P, T], BF16, name=f"ocat{p}", tag="ocat")
                for p in range(NPAIR)]

        prep_out = {}

        def prep_stream(p, w_sb, vt_slots=None):
            """q/k projection + rope for pair p, chunk-major (each x chunk
            is fully consumed before the next is touched, which keeps early
            PE demand under the DMA feed rate).  vt_slots optionally embeds
            v^T blocks after given chunks (pair 0: spreads the v work so it
            starts only once wv has landed).  Yields between PE-sized
            chunks; emission order sets scheduler priority so this work
            fills pair p-1's attention exp-wait bubbles."""
            qk = [qkpool.tile([P, T], BF16, tag="qkT", name="qkT")
                  for _ in range(2)]
            rots = [rotpool.tile([P, T], BF16, tag="rot", name="rot")
                    for _ in range(2)]
            for tc_i in range(NTC):
                sl = slice(tc_i * QW, (tc_i + 1) * QW)
                for jb in range(2):
                    dst = qk[jb]
                    bias_bc = bias_sb[
                        :, 2 * p + jb : 2 * p + jb + 1
                    ].to_broadcast((P, QW))
                    psum = ps_m.tile([P, QW], F32, tag="m", name="pj")
                    for cb in range(CB):
                        nc.tensor.matmul(
                            psum[:],
                            lhsT=w_sb[:, cb, jb * P : (jb + 1) * P],
                            rhs=xT_sb[tc_i][:, cb, :],
                            start=(cb == 0),
                            stop=(cb == CB - 1),
                        )
                    nc.vector.tensor_tensor(
                        dst[:, sl], psum[:], bias_bc, ALU.add,
                    )
                    yield
                for jb in range(2):  # rot = t*cos + P_swap @ (t*sin_signed)
                    src, rot = qk[jb], rots[jb]
                    sq = tmppool.tile([P, QW], BF16, tag="sq", name="sq")
                    nc.vector.tensor_tensor(
                        sq[:], src[:, sl], sin_sb[:, sl], ALU.mult
                    )
                    nc.vector.tensor_tensor(
                        rot[:, sl], src[:, sl], cos_sb[:, sl], ALU.mult
                    )
                    psum = ps_m.tile([P, QW], F32, tag="m", name="sw")
                    nc.tensor.matmul(
                        psum[:],
                        lhsT=psw_sb[:],
                        rhs=sq[:],
                        start=True,
                        stop=True,
                    )
                    nc.vector.tensor_tensor(
                        rot[:, sl], rot[:, sl], psum[:], ALU.add,
                    )
                    yield
                if vt_slots and tc_i in vt_slots:
                    lo, hi = vt_slots[tc_i]
                    for _ in vt_stream(lo, hi):
                        yield
            prep_out[p] = (rots[0], rots[1])

        def vt_stream(tb_lo, tb_hi):
            """v^T for 128-row blocks tb_lo..tb_hi-1, all pairs at once:
            psum[t, (p,h,d)] = sum_c x^T[c, t-block]^T wv^T[c, (p,h,d)]."""
            for tb in range(tb_lo, tb_hi):
                tci, loc = tb // 4, tb % 4
                psum = ps_m.tile([P, QW], F32, tag="m", name="vt")
                for cb in range(CB):
                    nc.tensor.matmul(
                        psum[:],
                        lhsT=xT_sb[tci][:, cb, loc * P : (loc + 1) * P],
                        rhs=wv_sb[:, cb, :],
                        start=(cb == 0),
                        stop=(cb == CB - 1),
                    )
                nc.vector.tensor_copy(
                    vn[:, tb, :, :, :HS],
                    psum[:].rearrange("p (pr h d) -> p pr h d", pr=NPAIR, h=2),
                )
                yield

        def emit_outproj_block(tb):
            """y[tb*P:(tb+1)*P, :] = sum_p ocat_p^T @ woT_p for one 128-row
            block; emitted as soon as all pairs' ocat columns are final.
            One row-contiguous DMA per block keeps the writeback (and the
            end-of-kernel DMA-semaphore sweep) at 16 transfers."""
            yb = ypool.tile([P, 2, QW], BF16, tag="yb")
            for oc in range(2):
                psum = ps_m.tile([P, QW], F32, tag="m", name="yp")
                for p in range(NPAIR):
                    nc.tensor.matmul(
                        psum[:],
                        lhsT=ocat[p][:, tb * P : (tb + 1) * P],
                        rhs=wo_sb[:, p, oc * QW : (oc + 1) * QW],
                        start=(p == 0),
                        stop=(p == NPAIR - 1),
                    )
                nc.vector.tensor_copy(yb[:, oc, :], psum[:])
            eng = nc.sync if tb % 2 == 0 else nc.gpsimd
            eng.dma_start(
                y_d[tb * P : (tb + 1) * P, :],
                yb[:].rearrange("p g q -> p (g q)"),
            )

        def attn_stream(p):
            """Attention for pair p.  Jobs are packed per (qc, kb): both
            heads' QK^T matmuls are emitted back-to-back (concurrent PE row
            tiles 0-63 / 64-127), followed by one fused exp over both heads'
            PSUM banks and the two AV accumulations.  Diagonal blocks only
            compute columns [off:]."""
            rq, rk = prep_out[p]
            ps_o_cur = {}

            def normalize(ext2, qc, heads):
                # 1/sums: ln+exp on the partition-64 sums row (same ACT
                # table as the scores exp -> no table reload)
                rcpf = rcppool.tile([65, 2, QW], F32, tag="rcpf", name="rcpf")
                nc.scalar.activation(
                    rcpf[64:65, heads, :], ext2[64:65, heads, :], AF.Ln
                )
                rcpb = rcppool.tile([65, 2, QW], BF16, tag="rcpb", name="rcpb")
                nc.scalar.activation(
                    rcpb[64:65, heads, :], rcpf[64:65, heads, :],
                    AF.Exp, scale=-1.0,
                )
                hs = range(2) if heads == slice(None) else [heads]
                for h in hs:
                    rb = ps_m.tile([HS, QW], F32, tag="m", name="rb")
                    nc.tensor.matmul(
                        rb[:],
                        lhsT=ones64[64:65, :],
                        rhs=rcpb[64:65, h, :],
                        start=True,
                        stop=True,
                    )
                    nc.vector.tensor_tensor(
                        ocat[p][h * HS : (h + 1) * HS,
                                qc * QW : (qc + 1) * QW],
                        ext2[:HS, h, :],
                        rb[:],
                        ALU.mult,
                    )

            for qc in range(NTC):
                nkb = 4 * (qc + 1)
                for h in range(2):
                    ps_o_cur[h] = ps_av.tile([HS + 1, QW], F32, tag="av",
                                             name="ps_o")
                for kb in range(nkb):
                    off = max(0, P * (kb - 4 * qc))
                    ps_s = ps_qk.tile([P, 2, QW], F32, tag="qk", name="ps_s")
                    for h in range(2):
                        nc.tensor.matmul(
                            ps_s[:, h, off:],
                            lhsT=rk[h * HS : (h + 1) * HS,
                                    kb * P : (kb + 1) * P],
                            rhs=rq[h * HS : (h + 1) * HS,
                                   qc * QW + off : (qc + 1) * QW],
                            start=True,
                            stop=True,
                        )
                    es = epool.tile([P, 2, QW], BF16, tag="es", name="es")
                    if off == 0:
                        nc.scalar.activation(
                            es[:].rearrange("p g q -> p (g q)"),
                            ps_s[:].rearrange("p g q -> p (g q)"),
                            AF.Exp,
                            scale=0.125,
                        )
                    else:  # diagonal block: partial exp + staircase mask
                        nc.scalar.activation(
                            es[:, :, off:], ps_s[:, :, off:],
                            AF.Exp, scale=0.125,
                        )
                    if kb - 4 * qc >= 0:
                        nc.vector.tensor_tensor(
                            es[:, :, off : off + P],
                            es[:, :, off : off + P],
                            band_sb[:],
                            ALU.mult,
                        )
                    # Late priority: when several PE instructions are ready
                    # the scheduler prefers QKT/prep work, so AVs trail exp
                    # by as much as the es pool depth allows (robust to
                    # cost-model vs hardware timing skew).
                    with tc.high_priority(offset=-1_000_000):
                        for h in range(2):
                            nc.tensor.matmul(
                                ps_o_cur[h][:, off:],
                                lhsT=vn[:, kb, p, h, :],
                                rhs=es[:, h, off:],
                                start=(kb == 0),
                                stop=(kb == nkb - 1),
                            )
                    yield
                # pair 3's normalize feeds the out-projection: keep it eager
                # and per-head there (shortest latency to the first rank-1);
                # defer it elsewhere (nothing reads ocat until pair 3)
                if p < NPAIR - 1:
                    # the copies free the ps_av banks for the next qc: keep
                    # them at normal priority; only the ln/exp/rank-1/mult
                    # (read ocat no earlier than pair 3) are deferred
                    ext2 = extpool.tile([HS + 1, 2, QW], F32, tag="ext",
                                        name="ext2")
                    for h in range(2):
                        nc.vector.tensor_copy(ext2[:, h, :],
                                              ps_o_cur[h][:])
                    with tc.high_priority(offset=-1_000_000):
                        normalize(ext2, qc, slice(None))
                else:
                    ext2 = extpool.tile([HS + 1, 2, QW], F32, tag="ext",
                                        name="ext2")
                    for h in range(2):
                        nc.vector.tensor_copy(ext2[:, h, :], ps_o_cur[h][:])
                        normalize(ext2, qc, h)
                yield
                if p == NPAIR - 1:
                    with tc.high_priority(offset=-1_000_000):
                        for tb in range(4 * qc, 4 * qc + 4):
                            emit_outproj_block(tb)
                    yield

        def drive(a_gen, b_gen, ratio=2):
            done_a = a_gen is None
            done_b = b_gen is None
            while not (done_a and done_b):
                if not done_a:
                    for _ in range(ratio):
                        try:
                            next(a_gen)
                        except StopIteration:
                            done_a = True
                            break
                if not done_b:
                    try:
                        next(b_gen)
                    except StopIteration:
                        done_b = True

        w_tiles = {0: w0_sb}
        w_tiles[1] = wpool.tile([P, CB, 2 * P], BF16, tag="w", name="w_sb")
        load_w(w_tiles[1], 1, engine=nc.scalar)
        # pair-0 prep embeds v^T blocks one chunk late (chunk tc carries
        # tb 4(tc-1)..4tc-1) so the first v^T lands after wv's DMA
        for _ in prep_stream(0, w0_sb,
                             vt_slots={1: (0, 4), 2: (4, 8), 3: (8, 12)}):
            pass
        for p in range(NPAIR):
            if p + 1 < NPAIR:
                if p + 2 < NPAIR:
                    w_tiles[p + 2] = wpool.tile([P, CB, 2 * P], BF16,
                                                tag="w", name="w_sb")
                    load_w(w_tiles[p + 2], p + 2)
                companion = prep_stream(p + 1, w_tiles[p + 1])
                if p == 0:
                    companion = chain(vt_stream(12, NKB), companion)
            else:
                companion = None
            drive(attn_stream(p), companion, ratio=1 if p == 0 else 2)
    return _legalize_waits(nc)


def _rope_tables():
    inv = 1.0 / (1000.0 ** (np.arange(0, HS, 2, dtype=np.float64) / HS))
    t = np.arange(T, dtype=np.float64)[:, None] * inv[None, :]
    sinT = np.sin(t).astype(np.float32).T  # [32, T]
    cosT = np.cos(t).astype(np.float32).T
    cosF = np.concatenate([cosT] * 4, 0)  # [128, T]
    # sign layout for multiply-BEFORE-swap: sq = q*sinF, swapped(sq) lands as
    # [-v*sin; +u*sin] in the [u; v] destination slots.
    sinF = np.concatenate([sinT, -sinT, sinT, -sinT], 0)
    return cosF, sinF


def _host_prep():
    cosF, sinF = _rope_tables()
    psw = np.zeros((P, P), np.float32)
    for hh in range(2):
        o = hh * HS
        psw[o : o + 32, o + 32 : o + 64] = np.eye(32)
        psw[o + 32 : o + 64, o : o + 32] = np.eye(32)
    # band[p, j] = 1 iff j >= p: causal triangle for the 128-wide diagonal
    # band, replicated for the two packed heads
    tri = np.tril(np.ones((P, P), np.float32)).T
    band = np.concatenate([tri, tri], axis=1)  # [P, 2P]
    return cosF, sinF, psw, band


def kernel(x, W_attn, b_attn, W_o, b_o, _trace=False, _tmpdir=None):
    x = np.asarray(x, np.float32)
    W_attn = np.asarray(W_attn, np.float32)
    b_attn = np.asarray(b_attn, np.float32)
    W_o = np.asarray(W_o, np.float32)
    b_o = np.asarray(b_o, np.float32)

    if "nc" not in _cache:
        _cache["nc"] = _build_nc()
    nc = _cache["nc"]

    cosF, sinF, psw, band = _host_prep()
    cosF_b, sinF_b = cosF.astype(NPBF), sinF.astype(NPBF)
    psw_b, band_b = psw.astype(NPBF), band.astype(NPBF)

    def head_rows(h):  # q-rows of head h, evens then odds
        base = h * HS
        return np.concatenate(
            [np.arange(base, base + HS, 2), np.arange(base + 1, base + HS, 2)]
        )

    in_maps = []
    for core in range(NCORES):
        b, hg = core // 2, core % 2
        heads = [hg * 8 + i for i in range(8)]
        qk_rows = []
        v_rows = []
        for p in range(NPAIR):
            h0, h1 = heads[2 * p], heads[2 * p + 1]
            qrows = np.concatenate([head_rows(h0), head_rows(h1)])
            qk_rows += [qrows, C + qrows]
            v_rows += [2 * C + np.concatenate(
                [np.arange(h0 * HS, (h0 + 1) * HS),
                 np.arange(h1 * HS, (h1 + 1) * HS)])]
        qk_rows = np.concatenate(qk_rows)  # [1024] pair-major (q,k) order
        v_rows = np.concatenate(v_rows)    # [512]  pair-major natural order
        # device-friendly swizzles: partition-major with contiguous per-
        # partition runs ([P, ...]) so DMAs move multi-KB packets
        wqkT = (
            W_attn[qk_rows].T.reshape(CB, P, NPAIR, 2 * P)
            .transpose(1, 2, 0, 3)
            .reshape(P, NPAIR * CB * 2 * P)
        )
        wqkT = np.ascontiguousarray(wqkT).astype(NPBF)
        bqk = np.ascontiguousarray(b_attn[qk_rows].reshape(8, P).T)  # [P, 8]
        wvT = (
            W_attn[v_rows].T.reshape(CB, P, 4 * P)
            .transpose(1, 0, 2)
            .reshape(P, CB * 4 * P)
        )
        wvT = np.ascontiguousarray(wvT).astype(NPBF)
        woT = (
            W_o[:, hg * 512 : (hg + 1) * 512].T.reshape(NPAIR, P, C)
            .transpose(1, 0, 2)
            .reshape(P, NPAIR * C)
        )
        woT = np.ascontiguousarray(woT).astype(NPBF)
        xT = (
            x[b].T.reshape(CB, P, NTC, QW)
            .transpose(1, 2, 0, 3)
            .reshape(P, NTC * CB * QW)
        )
        xT = np.ascontiguousarray(xT).astype(NPBF)
        in_maps.append(
            dict(xT=xT, wqkT=wqkT, bqk=bqk, wvT=wvT, woT=woT, cosF=cosF_b,
                 sinF=sinF_b, psw=psw_b, band=band_b)
        )

    res = run_bass_kernel_spmd(nc, in_maps, core_ids=list(range(NCORES)),
                               trace=_trace, tmpdir=_tmpdir)
    y = np.zeros((B, T, C), np.float32)
    for core in range(NCORES):
        y[core // 2] += res.results[core]["y"].astype(np.float32)
    # v-bias contribution: sum_k softmax_k (v_k + b_v) = (sum) + b_v, so
    # y gains the constant row b_v @ W_o^T; fold it in with b_o here.
    b_v = b_attn[2 * C :]
    y += (W_o @ b_v + b_o)[None, None, :]
    if _trace:
        _cache["last_result"] = res
    return y
